# revision 25
# baseline (speedup 1.0000x reference)
"""Trainium2 Bass kernel for nn_AdjacencyConv (GNN message passing).

Reference computation:
    msg  = relu(concat[x[src], x_bridge[bri]] @ lin_w.T + lin_b)   # [E, D]
    agg  = segment_sum(msg, dst, N)                                # [N, D]
    out  = agg + (1+eps)*x
    h    = relu(BN(out @ w1.T + b1)); h = relu(BN(h @ w2.T + b2))  # train-mode BN

Device algorithm (8-core SPMD, edges sharded by dst node-tile):
  Phase A (per core, replicated): build bf16 node tables in DRAM
        xw_tab = x @ Wx.T            (Wx = lin_w[:, :D])
        ew_tab = x_bridge @ Wb.T + b (Wb = lin_w[:, D:])
    so the per-edge linear factorizes: msg = relu(xw_tab[src] + ew_tab[bri]).
    The x-table is built first; the first KLA chunks' x-gathers are emitted
    before the e-table build so SWDGE descriptor generation (the kernel's
    bottleneck, ~3ns/idx serial on the GpSimd engine) overlaps phase A.
  Phase B: per dst node-tile of 128 nodes, per chunk of GCHUNK 128-edge
    batches: dma_gather x-rows and e-rows (bf16, 256B descriptors,
    4 SWDGE queues, x-gathers KLA chunks ahead) edge-major [128e, cb, D],
    add + relu in bf16, scatter-add via bf16 one-hot matmuls accumulating
    feature-major agg in PSUM (f32). Residual, linear1, per-tile BN stats
    and the h1 output DMA all run per-tile under later tiles' gathers.
  Phase C: BN stats are reduced across cores host-side between NEFFs
    (biases b1/b2 cancel in BN and are dropped; on-device AllReduce was
    measured ~170us per collective here — host reduction is faster).
    NEFF2 = BN1+relu+linear2, NEFF3 = BN2+relu+transpose to node-major.

Host side does only layout preprocessing: index sorting/padding/packing,
dtype casts, weight transposes, and output assembly.
"""

import os
import numpy as np
import ml_dtypes

BF16 = ml_dtypes.bfloat16

N, NB, E, D = 10000, 20000, 640000, 128
P = 128
NCORES = 8
NTC = 10                    # node tiles per core
N2 = NCORES * NTC * P       # 10240 padded nodes
NB2 = 20096                 # x_bridge padded to multiple of 128
R2 = N2 + NB2               # combined table rows
BN_EPS = 1e-5
GCHUNK = int(os.environ.get("KGCHUNK", "16"))  # batches per dma_gather
NQ = int(os.environ.get("KNQ", "4"))           # SWDGE queues
KSCRATCH = int(os.environ.get("KSCRATCH", "16384"))
KGB = int(os.environ.get("KGB", "3"))          # gather pool bufs
KFUSE = int(os.environ.get("KFUSE", "0"))      # 1 = single fused NEFF
KLA = int(os.environ.get("KLA", "10"))          # x-gather lookahead chunks
KP23 = int(os.environ.get("KP23", "1"))        # merged phase-2/3 NEFF

_cache = {}

# exposed for test.py
last_results = None


def _pack_idx(idx16):
    """Wrap an int16 index vector for dma_gather: [16, n/16] replicated x8."""
    w = idx16.reshape(-1, 16).T
    return np.tile(w, (8, 1)).copy()


def _host_prep(inputs):
    x = np.asarray(inputs["x"], np.float32)
    xb = np.asarray(inputs["x_bridge"], np.float32)
    ei = np.asarray(inputs["edge_index"])
    bri = np.asarray(inputs["bridge_index"])
    lin_w = np.asarray(inputs["lin_w"], np.float32)
    lin_b = np.asarray(inputs["lin_b"], np.float32)
    eps = float(np.asarray(inputs["eps"]).reshape(-1)[0])
    w1 = np.asarray(inputs["w1"], np.float32)
    g1 = np.asarray(inputs["g1"], np.float32)
    beta1 = np.asarray(inputs["beta1"], np.float32)
    w2 = np.asarray(inputs["w2"], np.float32)
    g2 = np.asarray(inputs["g2"], np.float32)
    beta2 = np.asarray(inputs["beta2"], np.float32)

    src = ei[0].astype(np.int64)
    dst = ei[1].astype(np.int64)
    bri = bri.astype(np.int64)

    # sort edges by dst, bucket into 128-node tiles
    order = np.argsort(dst, kind="stable")
    dsts = dst[order]
    srcs = src[order]
    bris = bri[order]
    gt_bounds = np.searchsorted(dsts, np.arange(NCORES * NTC + 1) * P)

    # uniform program structure: per local tile lt, same batch count across cores
    cnt = np.diff(gt_bounds)  # edges per global tile (len 80)
    cnt = cnt.reshape(NCORES, NTC)
    ceils = -(-cnt // P)
    B = np.maximum(1, ceils.max(axis=0))  # batches per local tile (len NTC)
    SB = int(B.sum())          # total batches per core
    S = SB * P                 # padded edges per core

    src_pad = np.zeros((NCORES, S), np.int64)
    bri_pad = np.zeros((NCORES, S), np.int64)
    dloc_pad = np.full((NCORES, S), 999.0, np.float32)
    for c in range(NCORES):
        off = 0
        for lt in range(NTC):
            gt = c * NTC + lt
            a, b = gt_bounds[gt], gt_bounds[gt + 1]
            n = b - a
            src_pad[c, off:off + n] = srcs[a:b]
            bri_pad[c, off:off + n] = bris[a:b]
            dloc_pad[c, off:off + n] = (dsts[a:b] - gt * P).astype(np.float32)
            off += B[lt] * P

    # dstloc transposed: [128, SB], column j = batch j's 128 local-dst values
    dloc_T = np.ascontiguousarray(
        dloc_pad.reshape(NCORES, SB, P).transpose(0, 2, 1))

    # feature-major bf16 padded inputs for the table build
    xt = np.zeros((D, N2), BF16)
    xt[:, :N] = x.T.astype(BF16)
    xbt = np.zeros((D, NB2), BF16)
    xbt[:, :NB] = xb.T.astype(BF16)

    wxt = np.ascontiguousarray(lin_w[:, :D].T).astype(BF16)   # [in_f, out]
    wbt = np.ascontiguousarray(lin_w[:, D:].T).astype(BF16)
    w1t = np.ascontiguousarray(w1.T)
    w2t = np.ascontiguousarray(w2.T)
    w2tb = w2t.astype(BF16)
    identb = np.eye(P, dtype=BF16)
    linbb = np.tile(lin_b[None, :], (P, 4)).astype(np.float32)   # [128, 512]
    iotab = np.tile(np.arange(P, dtype=np.float32)[None, :],
                    (P, GCHUNK)).astype(BF16)  # [128, GCHUNK*128]
    ident = np.eye(P, dtype=np.float32)

    g1c = np.ascontiguousarray(g1[:, None])
    b1c = np.ascontiguousarray(beta1[:, None])
    g2c = np.ascontiguousarray(g2[:, None])
    b2c = np.ascontiguousarray(beta2[:, None])

    # per-core residual slice (feature-major) and validity mask
    span = NTC * P
    xct = np.zeros((NCORES, D, span), np.float32)
    maskb = np.zeros((NCORES, P, span), np.float32)
    for c in range(NCORES):
        c0 = c * span
        v = min(max(N - c0, 0), span)
        if v > 0:
            xct[c, :, :v] = x.T[:, c0:c0 + v]
            maskb[c, :, :v] = 1.0

    in_maps = []
    for c in range(NCORES):
        in_maps.append({
            "xt": xt, "xbt": xbt,
            "wxt": wxt, "wbt": wbt, "w1t": w1t, "w2t": w2t,
            "w2tb": w2tb, "identb": identb,
            "linbb": linbb, "iotab": iotab, "ident": ident,
            "g1c": g1c, "b1c": b1c, "g2c": g2c, "b2c": b2c,
            "xct": np.ascontiguousarray(xct[c]),
            "maskb": np.ascontiguousarray(maskb[c]),
            "srcw": _pack_idx(src_pad[c].astype(np.int16)),
            "briw": _pack_idx(bri_pad[c].astype(np.int16)),
            "dstloc": np.ascontiguousarray(dloc_T[c].astype(BF16)),
        })
    meta = (tuple(int(b) for b in B), 1.0 + eps)
    return in_maps, meta


def _build(meta):
    import concourse.bacc as bacc
    import concourse.mybir as mybir
    import concourse.tile as tile

    B, resid_scale = meta
    SB = sum(B)
    S = SB * P
    f32 = mybir.dt.float32
    bf16 = mybir.dt.bfloat16
    i16 = mybir.dt.int16
    Alu = mybir.AluOpType
    Act = mybir.ActivationFunctionType
    span = NTC * P

    nc = bacc.Bacc("TRN2", target_bir_lowering=False, debug=False,
                   num_devices=NCORES, num_swdge_queues=NQ,
                   dynamic_dma_scratch_size=KSCRATCH)
    # Leave the top of SBUF for the runtime's SWDGE descriptor rings /
    # DynamicDMAScratch carveout — allocating into it wedges the device.
    nc.sbuf_top = min(nc.sbuf_top, 192 * 1024)

    def din(name, shape, dt=f32):
        return nc.dram_tensor(name, shape, dt, kind="ExternalInput")

    xt_d = din("xt", [D, N2], bf16); xbt_d = din("xbt", [D, NB2], bf16)
    wxt_d = din("wxt", [D, D], bf16); wbt_d = din("wbt", [D, D], bf16)
    w1t_d = din("w1t", [D, D])
    linbb_d = din("linbb", [P, 512]); iotab_d = din("iotab", [P, GCHUNK * P], bf16)
    xct_d = din("xct", [D, span])
    srcw_d = din("srcw", [128, S // 16], i16)
    briw_d = din("briw", [128, S // 16], i16)
    dstloc_d = din("dstloc", [P, SB], bf16)
    if KFUSE:
        w2t_d = din("w2t", [D, D])
        g1c_d = din("g1c", [P, 1]); b1c_d = din("b1c", [P, 1])
        g2c_d = din("g2c", [P, 1]); b2c_d = din("b2c", [P, 1])
        maskb_d = din("maskb", [P, span])
        ident_d = din("ident", [P, P])
        out_d = nc.dram_tensor("out", [span, D], f32, kind="ExternalOutput")
        bn_in = [nc.dram_tensor(f"bn_in{i}", [P, 2], f32) for i in range(2)]
        bn_out = [nc.dram_tensor(f"bn_out{i}", [P, 2], f32,
                                 addr_space="Shared") for i in range(2)]
    else:
        h_out_d = nc.dram_tensor("h_out", [P, span], f32, kind="ExternalOutput")
        stat_out_d = nc.dram_tensor("stat_out", [P, 2], f32,
                                    kind="ExternalOutput")

    xw_tab = nc.dram_tensor("xw_tab", [N2, D], bf16)
    ew_tab = nc.dram_tensor("ew_tab", [NB2, D], bf16)
    with tile.TileContext(nc) as tc:
        with (
            tc.tile_pool(name="consts", bufs=1) as cp,
            tc.tile_pool(name="pa_src", bufs=3) as pa_src,
            tc.tile_pool(name="pa_stg", bufs=3) as pa_stg,
            tc.tile_pool(name="psA", bufs=2, space="PSUM") as psA,
            tc.tile_pool(name="psB", bufs=2, space="PSUM") as psB,
            tc.tile_pool(name="gx", bufs=KLA + 2) as gxp,
            tc.tile_pool(name="ge", bufs=3) as gep,
            tc.tile_pool(name="oh", bufs=3) as ohp,
            tc.tile_pool(name="full", bufs=1) as fullp,
            tc.tile_pool(name="small", bufs=1) as smallp,
        ):
            def load_const(name, dram, shape, dt=f32):
                t = cp.tile(shape, dt, tag=f"c_{name}")
                nc.sync.dma_start(t[:], dram[:])
                return t

            wxt = load_const("wxt", wxt_d, [D, D], bf16)
            wbt = load_const("wbt", wbt_d, [D, D], bf16)
            w1t = load_const("w1t", w1t_d, [D, D])
            linbb = load_const("linbb", linbb_d, [P, 512])
            iotab = load_const("iotab", iotab_d, [P, GCHUNK * P], bf16)
            xct = load_const("xct", xct_d, [D, span])
            srcw = load_const("srcw", srcw_d, [128, S // 16], i16)
            briw = load_const("briw", briw_d, [128, S // 16], i16)
            dstloc = load_const("dstloc", dstloc_d, [P, SB], bf16)
            if KFUSE:
                w2t = load_const("w2t", w2t_d, [D, D])
                g1c = load_const("g1c", g1c_d, [P, 1])
                b1c = load_const("b1c", b1c_d, [P, 1])
                g2c = load_const("g2c", g2c_d, [P, 1])
                b2c = load_const("b2c", b2c_d, [P, 1])
                maskb = load_const("maskb", maskb_d, [P, span])
                ident = load_const("ident", ident_d, [P, P])

            # ---------------- Phase A: combined bf16 node table ----------------
            CW = 4096  # source columns per chunk

            def build_table(src_dram, ncols, w_sbuf, tab_dram, add_bias,
                            cw=CW):
                for c0 in range(0, ncols, cw):
                    w = min(cw, ncols - c0)
                    s = pa_src.tile([D, CW], bf16, tag="pa_src")
                    nc.sync.dma_start(s[:, :w], src_dram[:, c0:c0 + w])
                    g = pa_stg.tile([P, CW], bf16, tag="pa_stg")
                    for q0 in range(0, w, 512):
                        qw = min(512, w - q0)
                        ps = psA.tile([P, 512], f32, tag="psAb")
                        for j in range(qw // P):
                            nc.tensor.matmul(
                                ps[:, j * P:(j + 1) * P],
                                s[:, q0 + j * P:q0 + (j + 1) * P], w_sbuf[:])
                        sl = g[:, q0:q0 + qw]
                        if add_bias:
                            nc.vector.tensor_tensor(sl, ps[:, :qw],
                                                    linbb[:, :qw], Alu.add)
                        else:
                            nc.scalar.activation(sl, ps[:, :qw], Act.Copy)
                    nc.scalar.dma_start(
                        tab_dram[c0:c0 + w, :].rearrange(
                            "(g p) d -> p g d", p=P),
                        g[:, :w].rearrange("p (g d) -> p g d", d=P))

            # ---------------- Phase B: gather + scatter-add ----------------
            # chunk list: (lt, batch_off, cb, first_of_tile, last_of_tile)
            chunks = []
            prefix = [0]
            for t in range(NTC):
                prefix.append(prefix[-1] + B[t])
            for lt in sorted(range(NTC), key=lambda t: -B[t]):
                nb = B[lt]
                done = 0
                while done < nb:
                    cb = min(GCHUNK, nb - done)
                    chunks.append((lt, prefix[lt] + done, cb, done == 0,
                                   done + cb == nb))
                    done += cb

            outT = fullp.tile([P, span], f32, tag="outT")
            h1 = fullp.tile([P, span], f32, tag="h")
            s_parts = smallp.tile([P, 2 * NTC], f32, tag="s_parts")
            gq = [0]
            gx_tiles = {}

            def emit_gx(j):
                lt, boff, cb, _, _ = chunks[j]
                ne = cb * P
                gx = gxp.tile([P, GCHUNK, D], bf16, tag="gx")
                nc.gpsimd.dma_gather(
                    gx[:, :cb, :], xw_tab[:],
                    srcw[:, boff * 8:boff * 8 + ne // 16],
                    ne, ne, D, single_packet=False, queue_num=gq[0] % NQ)
                gq[0] += 1
                gx_tiles[j] = gx

            # zero-fill the gather pool buffers once so trailing slots of
            # negative-trimmed (padded) indices stay finite (one-hot zeroes
            # their contribution; memsets run under phase A)
            for _ in range(KLA + 2):
                t = gxp.tile([P, GCHUNK, D], bf16, tag="gx")
                nc.vector.memset(t[:], 0.0)
            for _ in range(3):
                t = gep.tile([P, GCHUNK, D], bf16, tag="ge")
                nc.vector.memset(t[:], 0.0)

            # x-table, then early x-gathers overlap the e-table build
            build_table(xt_d, N2, wxt, xw_tab, False, cw=2048)
            LA = min(KLA, len(chunks))
            for j in range(LA):
                emit_gx(j)
            build_table(xbt_d, NB2, wbt, ew_tab, True)

            aggT = None
            for j, (lt, boff, cb, first_c, last_c) in enumerate(chunks):
                ne = cb * P
                if first_c:
                    aggT = psB.tile([P, P], f32, tag="aggT")
                ge = gep.tile([P, GCHUNK, D], bf16, tag="ge")
                nc.gpsimd.dma_gather(
                    ge[:, :cb, :], ew_tab[:],
                    briw[:, boff * 8:boff * 8 + ne // 16],
                    ne, ne, D, single_packet=False, queue_num=gq[0] % NQ)
                gq[0] += 1
                if j + LA < len(chunks):
                    emit_gx(j + LA)
                gx = gx_tiles.pop(j)

                nc.vector.tensor_tensor(gx[:, :cb, :], gx[:, :cb, :],
                                        ge[:, :cb, :], Alu.add)
                nc.scalar.activation(gx[:, :cb, :], gx[:, :cb, :], Act.Relu)

                oh = ohp.tile([P, GCHUNK * P], bf16, tag="oh")
                dl = dstloc[:, boff:boff + cb]
                dl_rep = dl.to_broadcast((P, cb, P))
                nc.vector.tensor_tensor(
                    oh[:, :cb * P].rearrange("p (b c) -> p b c", c=P),
                    iotab[:, :cb * P].rearrange("p (b c) -> p b c", c=P),
                    dl_rep, Alu.is_equal)
                for b in range(cb):
                    nc.tensor.matmul(
                        aggT[:], gx[:, b, :], oh[:, b * P:(b + 1) * P],
                        start=(first_c and b == 0), stop=(last_c and b == cb - 1))

                if last_c:
                    # residual + linear1 + stats + h1-out for this tile,
                    # all under later tiles' gathers
                    sl = slice(lt * P, (lt + 1) * P)
                    nc.vector.scalar_tensor_tensor(
                        outT[:, sl], xct[:, sl], float(resid_scale), aggT[:],
                        Alu.mult, Alu.add)
                    ps2 = psA.tile([P, P], f32, tag="psA")
                    nc.tensor.matmul(ps2[:], w1t[:], outT[:, sl])
                    nc.scalar.activation(h1[:, sl], ps2[:], Act.Copy)
                    nc.vector.reduce_sum(s_parts[:, 2 * lt:2 * lt + 1],
                                         h1[:, sl], axis=mybir.AxisListType.X)
                    sqt = smallp.tile([P, P], f32, tag="sqt")
                    nc.vector.tensor_tensor(sqt[:], h1[:, sl], h1[:, sl],
                                            Alu.mult)
                    nc.vector.reduce_sum(s_parts[:, 2 * lt + 1:2 * lt + 2],
                                         sqt[:], axis=mybir.AxisListType.X)
                    nc.sync.dma_start(h_out_d[:, sl], h1[:, sl])

            # ---------------- Phase C head: per-core BN stats --------
            s_stat = smallp.tile([P, 2], f32, tag="stat1")
            nc.vector.reduce_sum(
                s_stat[:, 0:1],
                s_parts[:].rearrange("p (t two) -> p t two", two=2)[:, :, 0:1],
                axis=mybir.AxisListType.XY)
            nc.vector.reduce_sum(
                s_stat[:, 1:2],
                s_parts[:].rearrange("p (t two) -> p t two", two=2)[:, :, 1:2],
                axis=mybir.AxisListType.XY)
            if not KFUSE:
                nc.sync.dma_start(stat_out_d[:], s_stat[:])
            else:
                # ---- fused tail: BN1 -> relu -> mask -> linear2 -> BN2 ----
                def allreduce_stats(idx, stat_tile):
                    nc.sync.dma_start(bn_in[idx][:], stat_tile[:])
                    nc.gpsimd.collective_compute(
                        "AllReduce", Alu.add,
                        replica_groups=[list(range(NCORES))],
                        ins=[bn_in[idx][:]], outs=[bn_out[idx][:]])
                    red = smallp.tile([P, 2], f32, tag=f"red{idx}")
                    nc.sync.dma_start(red[:], bn_out[idx][:])
                    return red

                red0 = allreduce_stats(0, s_stat)
                a0, sh0 = _bn_scale_shift(nc, mybir, smallp, red0, g1c, b1c, 0)
                # BN1-relu in place on h1, then mask
                nc.scalar.activation(h1[:], h1[:], Act.Relu,
                                     bias=sh0[:], scale=a0[:])
                nc.vector.tensor_tensor(h1[:], h1[:], maskb[:], Alu.mult)

                h2 = fullp.tile([P, span], f32, tag="h2")
                for lt in range(NTC):
                    sl = slice(lt * P, (lt + 1) * P)
                    ps = psA.tile([P, P], f32, tag="psA")
                    nc.tensor.matmul(ps[:], w2t[:], h1[:, sl])
                    nc.scalar.activation(h2[:, sl], ps[:], Act.Copy)
                s2 = smallp.tile([P, 2], f32, tag="stat2")
                nc.vector.reduce_sum(s2[:, 0:1], h2[:],
                                     axis=mybir.AxisListType.X)
                sq2 = fullp.tile([P, span], f32, tag="sq")
                nc.vector.tensor_tensor(sq2[:], h2[:], h2[:], Alu.mult)
                nc.vector.reduce_sum(s2[:, 1:2], sq2[:],
                                     axis=mybir.AxisListType.X)
                red1 = allreduce_stats(1, s2)
                a1, sh1 = _bn_scale_shift(nc, mybir, smallp, red1, g2c, b2c, 1)
                nc.scalar.activation(h2[:], h2[:], Act.Relu,
                                     bias=sh1[:], scale=a1[:])

                stg = fullp.tile([P, span], f32, tag="sq")
                for lt in range(NTC):
                    sl = slice(lt * P, (lt + 1) * P)
                    ps = psA.tile([P, P], f32, tag="psA")
                    nc.tensor.transpose(ps[:], h2[:, sl], ident[:])
                    nc.scalar.activation(stg[:, sl], ps[:], Act.Copy)
                nc.sync.dma_start(
                    out_d[:].rearrange("(g p) d -> p g d", p=P),
                    stg[:].rearrange("p (g d) -> p g d", d=P))

    nc.compile()
    return nc


_cache2 = {}


def _bn_scale_shift(nc, mybir, smallp, red, gcol, bcol, idx):
    """Device-side BN coefficients from reduced stats: a = g*rstd, sh = b - mu*a."""
    f32 = mybir.dt.float32
    Alu = mybir.AluOpType
    Act = mybir.ActivationFunctionType
    mu = smallp.tile([P, 1], f32, tag=f"mu{idx}")
    nc.vector.tensor_scalar(mu[:], red[:, 0:1], 1.0 / N, None, Alu.mult)
    mu2 = smallp.tile([P, 1], f32, tag=f"mu2{idx}")
    nc.vector.tensor_tensor(mu2[:], mu[:], mu[:], Alu.mult)
    e2 = smallp.tile([P, 1], f32, tag=f"e2{idx}")
    nc.vector.tensor_scalar(e2[:], red[:, 1:2], 1.0 / N, None, Alu.mult)
    var = smallp.tile([P, 1], f32, tag=f"var{idx}")
    nc.vector.tensor_tensor(var[:], e2[:], mu2[:], Alu.subtract)
    vep = smallp.tile([P, 1], f32, tag=f"vep{idx}")
    nc.vector.tensor_scalar(vep[:], var[:], BN_EPS, None, Alu.add)
    std = smallp.tile([P, 1], f32, tag=f"std{idx}")
    nc.scalar.activation(std[:], vep[:], Act.Sqrt)
    rstd = smallp.tile([P, 1], f32, tag=f"rstd{idx}")
    nc.vector.reciprocal(rstd[:], std[:])
    a = smallp.tile([P, 1], f32, tag=f"a{idx}")
    nc.vector.tensor_tensor(a[:], gcol[:], rstd[:], Alu.mult)
    tmp = smallp.tile([P, 1], f32, tag=f"tmp{idx}")
    nc.vector.tensor_tensor(tmp[:], mu[:], a[:], Alu.mult)
    sh = smallp.tile([P, 1], f32, tag=f"sh{idx}")
    nc.vector.tensor_tensor(sh[:], bcol[:], tmp[:], Alu.subtract)
    return a, sh


def _build_phase2():
    """NEFF2: h1n = mask*relu(BN1(h1)); h2 = h1n @ w2.T; per-core stats of h2."""
    import concourse.bacc as bacc
    import concourse.mybir as mybir
    import concourse.tile as tile

    f32 = mybir.dt.float32
    Alu = mybir.AluOpType
    Act = mybir.ActivationFunctionType
    span = NTC * P

    nc = bacc.Bacc("TRN2", target_bir_lowering=False, debug=False,
                   num_devices=NCORES)
    nc.sbuf_top = min(nc.sbuf_top, 192 * 1024)

    def din(name, shape):
        return nc.dram_tensor(name, shape, f32, kind="ExternalInput")

    h_d = din("h_in", [P, span])
    red_d = din("red", [P, 2])
    w2t_d = din("w2t", [D, D])
    g1c_d = din("g1c", [P, 1]); b1c_d = din("b1c", [P, 1])
    maskb_d = din("maskb", [P, span])
    h_out_d = nc.dram_tensor("h_out", [P, span], f32, kind="ExternalOutput")
    stat_out_d = nc.dram_tensor("stat_out", [P, 2], f32, kind="ExternalOutput")

    with tile.TileContext(nc) as tc:
        with (
            tc.tile_pool(name="consts", bufs=1) as cp,
            tc.tile_pool(name="psA", bufs=2, space="PSUM") as psA,
            tc.tile_pool(name="full", bufs=1) as fullp,
            tc.tile_pool(name="small", bufs=1) as smallp,
        ):
            def load_const(name, dram, shape):
                t = cp.tile(shape, f32, tag=f"c_{name}")
                nc.sync.dma_start(t[:], dram[:])
                return t

            h1 = load_const("h", h_d, [P, span])
            red = load_const("red", red_d, [P, 2])
            w2t = load_const("w2t", w2t_d, [D, D])
            g1c = load_const("g1c", g1c_d, [P, 1])
            b1c = load_const("b1c", b1c_d, [P, 1])
            maskb = load_const("maskb", maskb_d, [P, span])

            a, sh = _bn_scale_shift(nc, mybir, smallp, red, g1c, b1c, 0)
            hn = fullp.tile([P, span], f32, tag="hn")
            nc.scalar.activation(hn[:], h1[:], Act.Relu, bias=sh[:], scale=a[:])
            nc.vector.tensor_tensor(hn[:], hn[:], maskb[:], Alu.mult)

            h2 = fullp.tile([P, span], f32, tag="h2")
            for lt in range(NTC):
                sl = slice(lt * P, (lt + 1) * P)
                ps = psA.tile([P, P], f32, tag="psA")
                nc.tensor.matmul(ps[:], w2t[:], hn[:, sl])
                nc.scalar.activation(h2[:, sl], ps[:], Act.Copy)
            s_stat = smallp.tile([P, 2], f32, tag="stat2")
            nc.vector.reduce_sum(s_stat[:, 0:1], h2[:],
                                 axis=mybir.AxisListType.X)
            sq = fullp.tile([P, span], f32, tag="sq")
            nc.vector.tensor_tensor(sq[:], h2[:], h2[:], Alu.mult)
            nc.vector.reduce_sum(s_stat[:, 1:2], sq[:],
                                 axis=mybir.AxisListType.X)
            nc.sync.dma_start(stat_out_d[:], s_stat[:])
            nc.sync.dma_start(h_out_d[:], h2[:])

    nc.compile()
    return nc


def _build_phase3():
    """NEFF3: out = transpose(relu(BN2(h2)))."""
    import concourse.bacc as bacc
    import concourse.mybir as mybir
    import concourse.tile as tile

    f32 = mybir.dt.float32
    Act = mybir.ActivationFunctionType
    span = NTC * P

    nc = bacc.Bacc("TRN2", target_bir_lowering=False, debug=False,
                   num_devices=NCORES)
    nc.sbuf_top = min(nc.sbuf_top, 192 * 1024)

    def din(name, shape):
        return nc.dram_tensor(name, shape, f32, kind="ExternalInput")

    h_d = din("h_in", [P, span])
    red_d = din("red", [P, 2])
    g2c_d = din("g2c", [P, 1]); b2c_d = din("b2c", [P, 1])
    ident_d = din("ident", [P, P])
    out_d = nc.dram_tensor("out", [span, D], f32, kind="ExternalOutput")

    with tile.TileContext(nc) as tc:
        with (
            tc.tile_pool(name="consts", bufs=1) as cp,
            tc.tile_pool(name="psA", bufs=2, space="PSUM") as psA,
            tc.tile_pool(name="full", bufs=1) as fullp,
            tc.tile_pool(name="small", bufs=1) as smallp,
        ):
            def load_const(name, dram, shape):
                t = cp.tile(shape, f32, tag=f"c_{name}")
                nc.sync.dma_start(t[:], dram[:])
                return t

            h2 = load_const("h", h_d, [P, span])
            red = load_const("red", red_d, [P, 2])
            g2c = load_const("g2c", g2c_d, [P, 1])
            b2c = load_const("b2c", b2c_d, [P, 1])
            ident = load_const("ident", ident_d, [P, P])

            a, sh = _bn_scale_shift(nc, mybir, smallp, red, g2c, b2c, 1)
            hn = fullp.tile([P, span], f32, tag="hn")
            nc.scalar.activation(hn[:], h2[:], Act.Relu, bias=sh[:], scale=a[:])

            stg = fullp.tile([P, span], f32, tag="stg")
            for lt in range(NTC):
                sl = slice(lt * P, (lt + 1) * P)
                ps = psA.tile([P, P], f32, tag="psA")
                nc.tensor.transpose(ps[:], hn[:, sl], ident[:])
                nc.scalar.activation(stg[:, sl], ps[:], Act.Copy)
            nc.sync.dma_start(
                out_d[:].rearrange("(g p) d -> p g d", p=P),
                stg[:].rearrange("p (g d) -> p g d", d=P))

    nc.compile()
    return nc


def _build_phase23():
    """Single second NEFF: every core gets the full (rotated) bf16 h1 with
    its own span at columns [0:span] and exact zeros appended for the 240
    padded nodes. BN1-apply + linear2 + global BN2 stats are computed
    replicated (mask-free, exact); BN2-apply + transpose + output only for
    the own span. Removes one NEFF launch and one host sync."""
    import concourse.bacc as bacc
    import concourse.mybir as mybir
    import concourse.tile as tile

    f32 = mybir.dt.float32
    bf16 = mybir.dt.bfloat16
    Alu = mybir.AluOpType
    Act = mybir.ActivationFunctionType
    span = NTC * P
    FULL = N2  # 10240 = 10000 real + 240 zero columns

    nc = bacc.Bacc("TRN2", target_bir_lowering=False, debug=False,
                   num_devices=NCORES)
    nc.sbuf_top = min(nc.sbuf_top, 192 * 1024)

    h_d = nc.dram_tensor("h_all", [P, FULL], bf16, kind="ExternalInput")
    red_d = nc.dram_tensor("red", [P, 2], f32, kind="ExternalInput")
    w2tb_d = nc.dram_tensor("w2tb", [D, D], bf16, kind="ExternalInput")
    g1c_d = nc.dram_tensor("g1c", [P, 1], f32, kind="ExternalInput")
    b1c_d = nc.dram_tensor("b1c", [P, 1], f32, kind="ExternalInput")
    g2c_d = nc.dram_tensor("g2c", [P, 1], f32, kind="ExternalInput")
    b2c_d = nc.dram_tensor("b2c", [P, 1], f32, kind="ExternalInput")
    ident_d = nc.dram_tensor("identb", [P, P], bf16, kind="ExternalInput")
    out_d = nc.dram_tensor("out", [span, D], f32, kind="ExternalOutput")

    with tile.TileContext(nc) as tc:
        with (
            tc.tile_pool(name="consts", bufs=1) as cp,
            tc.tile_pool(name="ps", bufs=2, space="PSUM") as psp,
            tc.tile_pool(name="full", bufs=1) as fullp,
            tc.tile_pool(name="small", bufs=1) as smallp,
        ):
            def load_const(name, dram, shape, dt=f32):
                t = cp.tile(shape, dt, tag=f"c_{name}")
                nc.sync.dma_start(t[:], dram[:])
                return t

            h1 = load_const("h", h_d, [P, FULL], bf16)
            red = load_const("red", red_d, [P, 2])
            w2tb = load_const("w2tb", w2tb_d, [D, D], bf16)
            g1c = load_const("g1c", g1c_d, [P, 1])
            b1c = load_const("b1c", b1c_d, [P, 1])
            g2c = load_const("g2c", g2c_d, [P, 1])
            b2c = load_const("b2c", b2c_d, [P, 1])
            ident = load_const("ident", ident_d, [P, P], bf16)

            a0, sh0 = _bn_scale_shift(nc, mybir, smallp, red, g1c, b1c, 0)
            # BN1-apply + relu in place (bf16); zero columns stay relu(sh0)
            # only in the 240 appended pad columns... they must be zeroed
            # for exact stats: recompute them as zeros via memset after.
            nc.scalar.activation(h1[:], h1[:], Act.Relu,
                                 bias=sh0[:], scale=a0[:])
            nc.vector.memset(h1[:, N:FULL], 0.0)

            # linear2 over the full width, bf16, w2 stationary
            h2 = fullp.tile([P, FULL], bf16, tag="h2")
            for q0 in range(0, FULL, 512):
                ps = psp.tile([P, 512], f32, tag="ps")
                for j in range(4):
                    nc.tensor.matmul(ps[:, j * P:(j + 1) * P], w2tb[:],
                                     h1[:, q0 + j * P:q0 + (j + 1) * P])
                nc.scalar.activation(h2[:, q0:q0 + 512], ps[:], Act.Copy)

            # global BN2 stats (exact: pad columns of h2 = W2 @ 0 = 0)
            s2 = smallp.tile([P, 2], f32, tag="s2")
            nc.vector.reduce_sum(s2[:, 0:1], h2[:], axis=mybir.AxisListType.X)
            sq = fullp.tile([P, FULL], f32, tag="sq")
            nc.vector.tensor_tensor(sq[:], h2[:], h2[:], Alu.mult)
            nc.vector.reduce_sum(s2[:, 1:2], sq[:], axis=mybir.AxisListType.X)
            a1, sh1 = _bn_scale_shift(nc, mybir, smallp, s2, g2c, b2c, 1)

            # BN2-apply + relu + transpose for the own span only
            hn2 = fullp.tile([P, span], bf16, tag="hn2")
            nc.scalar.activation(hn2[:], h2[:, :span], Act.Relu,
                                 bias=sh1[:], scale=a1[:])
            stg = fullp.tile([P, span], f32, tag="stg")
            for lt in range(NTC):
                sl = slice(lt * P, (lt + 1) * P)
                ps = psp.tile([P, P], bf16, tag="pst")
                nc.tensor.transpose(ps[:], hn2[:, sl], ident[:])
                nc.scalar.activation(stg[:, sl], ps[:], Act.Copy)
            nc.sync.dma_start(
                out_d[:].rearrange("(g p) d -> p g d", p=P),
                stg[:].rearrange("p (g d) -> p g d", d=P))

    nc.compile()
    return nc


def kernel(**inputs):
    global last_results
    from concourse.bass_utils import run_bass_kernel_spmd

    in_maps, meta = _host_prep(inputs)
    if meta not in _cache:
        _cache[meta] = _build(meta)
    if not KFUSE and KP23 and "p23" not in _cache2:
        _cache2["p23"] = _build_phase23()
    if not KFUSE and not KP23 and "p2" not in _cache2:
        _cache2["p2"] = _build_phase2()
        _cache2["p3"] = _build_phase3()
    nc1 = _cache[meta]
    nc2 = _cache2.get("p2"); nc3 = _cache2.get("p3")
    cores = list(range(NCORES))
    trace = bool(os.environ.get("KERNEL_TRACE"))

    n1_keys = ["xt", "xbt", "wxt", "wbt", "w1t", "linbb", "iotab",
               "xct", "srcw", "briw", "dstloc"]
    if KFUSE:
        n1_keys += ["w2t", "g1c", "b1c", "g2c", "b2c", "maskb", "ident"]
        in1 = [{k: in_maps[c][k] for k in n1_keys} for c in range(NCORES)]
        res1 = run_bass_kernel_spmd(nc1, in1, cores, trace=trace)
        last_results = (res1,)
        out = np.concatenate([res1.results[c]["out"] for c in range(NCORES)],
                             axis=0)
        return np.ascontiguousarray(out[:N])
    in1 = [{k: in_maps[c][k] for k in n1_keys} for c in range(NCORES)]
    res1 = run_bass_kernel_spmd(nc1, in1, cores, trace=trace)
    red1 = np.sum([res1.results[c]["stat_out"] for c in range(NCORES)], axis=0)
    if KP23:
        span = NTC * P
        hb = np.concatenate([res1.results[c]["h_out"] for c in range(NCORES)],
                            axis=1).astype(BF16)  # [P, 10240]
        zpad = np.zeros((P, N2 - N), BF16)
        in2 = []
        for c in range(NCORES):
            roll = np.concatenate(
                [hb[:, span * c:N], hb[:, :span * c], zpad], axis=1)
            in2.append({"h_all": np.ascontiguousarray(roll), "red": red1,
                        "w2tb": in_maps[c]["w2tb"],
                        "identb": in_maps[c]["identb"],
                        "g1c": in_maps[c]["g1c"], "b1c": in_maps[c]["b1c"],
                        "g2c": in_maps[c]["g2c"], "b2c": in_maps[c]["b2c"]})
        res2 = run_bass_kernel_spmd(_cache2["p23"], in2, cores, trace=trace)
        last_results = (res1, res2)
        out = np.concatenate([res2.results[c]["out"] for c in range(NCORES)],
                             axis=0)
        return np.ascontiguousarray(out[:N])
    in2 = [{"h_in": res1.results[c]["h_out"], "red": red1,
            "w2t": in_maps[c]["w2t"], "g1c": in_maps[c]["g1c"],
            "b1c": in_maps[c]["b1c"], "maskb": in_maps[c]["maskb"]}
           for c in range(NCORES)]
    res2 = run_bass_kernel_spmd(nc2, in2, cores, trace=trace)
    red2 = np.sum([res2.results[c]["stat_out"] for c in range(NCORES)], axis=0)
    in3 = [{"h_in": res2.results[c]["h_out"], "red": red2,
            "g2c": in_maps[c]["g2c"], "b2c": in_maps[c]["b2c"],
            "ident": in_maps[c]["ident"]} for c in range(NCORES)]
    res3 = run_bass_kernel_spmd(nc3, in3, cores, trace=trace)

    last_results = (res1, res2, res3)
    out = np.concatenate([res3.results[c]["out"] for c in range(NCORES)], axis=0)
    return np.ascontiguousarray(out[:N])


# revision 26
# speedup vs baseline: 1.0568x; 1.0568x over previous
"""Trainium2 Bass kernel for nn_AdjacencyConv (GNN message passing).

Reference computation:
    msg  = relu(concat[x[src], x_bridge[bri]] @ lin_w.T + lin_b)   # [E, D]
    agg  = segment_sum(msg, dst, N)                                # [N, D]
    out  = agg + (1+eps)*x
    h    = relu(BN(out @ w1.T + b1)); h = relu(BN(h @ w2.T + b2))  # train-mode BN

Device algorithm (8-core SPMD, edges sharded by dst node-tile):
  Phase A (per core, replicated): build bf16 node tables in DRAM
        xw_tab = x @ Wx.T            (Wx = lin_w[:, :D])
        ew_tab = x_bridge @ Wb.T + b (Wb = lin_w[:, D:])
    so the per-edge linear factorizes: msg = relu(xw_tab[src] + ew_tab[bri]).
    The x-table is built first; the first KLA chunks' x-gathers are emitted
    before the e-table build so SWDGE descriptor generation (the kernel's
    bottleneck, ~3ns/idx serial on the GpSimd engine) overlaps phase A.
  Phase B: per dst node-tile of 128 nodes, per chunk of GCHUNK 128-edge
    batches: dma_gather x-rows and e-rows (bf16, 256B descriptors,
    4 SWDGE queues, x-gathers KLA chunks ahead) edge-major [128e, cb, D],
    add + relu in bf16, scatter-add via bf16 one-hot matmuls accumulating
    feature-major agg in PSUM (f32). Residual, linear1, per-tile BN stats
    and the h1 output DMA all run per-tile under later tiles' gathers.
  Phase C: BN stats are reduced across cores host-side between NEFFs
    (biases b1/b2 cancel in BN and are dropped; on-device AllReduce was
    measured ~170us per collective here — host reduction is faster).
    NEFF2 = BN1+relu+linear2, NEFF3 = BN2+relu+transpose to node-major.

Host side does only layout preprocessing: index sorting/padding/packing,
dtype casts, weight transposes, and output assembly.
"""

import os
import numpy as np
import ml_dtypes

BF16 = ml_dtypes.bfloat16

N, NB, E, D = 10000, 20000, 640000, 128
P = 128
NCORES = 8
NTC = 10                    # node tiles per core
N2 = NCORES * NTC * P       # 10240 padded nodes
NB2 = 20096                 # x_bridge padded to multiple of 128
R2 = N2 + NB2               # combined table rows
BN_EPS = 1e-5
GCHUNK = int(os.environ.get("KGCHUNK", "16"))  # batches per dma_gather
NQ = int(os.environ.get("KNQ", "4"))           # SWDGE queues
KSCRATCH = int(os.environ.get("KSCRATCH", "16384"))
KGB = int(os.environ.get("KGB", "3"))          # gather pool bufs
KFUSE = int(os.environ.get("KFUSE", "0"))      # 1 = single fused NEFF
KLA = int(os.environ.get("KLA", "10"))          # x-gather lookahead chunks
KP23 = int(os.environ.get("KP23", "0"))        # merged phase-2/3 NEFF

_cache = {}

# exposed for test.py
last_results = None


def _pack_idx(idx16):
    """Wrap an int16 index vector for dma_gather: [16, n/16] replicated x8."""
    w = idx16.reshape(-1, 16).T
    return np.tile(w, (8, 1)).copy()


def _host_prep(inputs):
    x = np.asarray(inputs["x"], np.float32)
    xb = np.asarray(inputs["x_bridge"], np.float32)
    ei = np.asarray(inputs["edge_index"])
    bri = np.asarray(inputs["bridge_index"])
    lin_w = np.asarray(inputs["lin_w"], np.float32)
    lin_b = np.asarray(inputs["lin_b"], np.float32)
    eps = float(np.asarray(inputs["eps"]).reshape(-1)[0])
    w1 = np.asarray(inputs["w1"], np.float32)
    g1 = np.asarray(inputs["g1"], np.float32)
    beta1 = np.asarray(inputs["beta1"], np.float32)
    w2 = np.asarray(inputs["w2"], np.float32)
    g2 = np.asarray(inputs["g2"], np.float32)
    beta2 = np.asarray(inputs["beta2"], np.float32)

    src = ei[0].astype(np.int64)
    dst = ei[1].astype(np.int64)
    bri = bri.astype(np.int64)

    # sort edges by dst, bucket into 128-node tiles
    order = np.argsort(dst, kind="stable")
    dsts = dst[order]
    srcs = src[order]
    bris = bri[order]
    gt_bounds = np.searchsorted(dsts, np.arange(NCORES * NTC + 1) * P)

    # uniform program structure: per local tile lt, same batch count across cores
    cnt = np.diff(gt_bounds)  # edges per global tile (len 80)
    cnt = cnt.reshape(NCORES, NTC)
    ceils = -(-cnt // P)
    B = np.maximum(1, ceils.max(axis=0))  # batches per local tile (len NTC)
    SB = int(B.sum())          # total batches per core
    S = SB * P                 # padded edges per core

    src_pad = np.zeros((NCORES, S), np.int64)
    bri_pad = np.zeros((NCORES, S), np.int64)
    dloc_pad = np.full((NCORES, S), 999.0, np.float32)
    for c in range(NCORES):
        off = 0
        for lt in range(NTC):
            gt = c * NTC + lt
            a, b = gt_bounds[gt], gt_bounds[gt + 1]
            n = b - a
            src_pad[c, off:off + n] = srcs[a:b]
            bri_pad[c, off:off + n] = bris[a:b]
            dloc_pad[c, off:off + n] = (dsts[a:b] - gt * P).astype(np.float32)
            off += B[lt] * P

    # dstloc transposed: [128, SB], column j = batch j's 128 local-dst values
    dloc_T = np.ascontiguousarray(
        dloc_pad.reshape(NCORES, SB, P).transpose(0, 2, 1))

    # feature-major bf16 padded inputs for the table build
    xt = np.zeros((D, N2), BF16)
    xt[:, :N] = x.T.astype(BF16)
    xbt = np.zeros((D, NB2), BF16)
    xbt[:, :NB] = xb.T.astype(BF16)

    wxt = np.ascontiguousarray(lin_w[:, :D].T).astype(BF16)   # [in_f, out]
    wbt = np.ascontiguousarray(lin_w[:, D:].T).astype(BF16)
    w1t = np.ascontiguousarray(w1.T)
    w2t = np.ascontiguousarray(w2.T)
    w2tb = w2t.astype(BF16)
    identb = np.eye(P, dtype=BF16)
    linbb = np.tile(lin_b[None, :], (P, 4)).astype(np.float32)   # [128, 512]
    iotab = np.tile(np.arange(P, dtype=np.float32)[None, :],
                    (P, GCHUNK)).astype(BF16)  # [128, GCHUNK*128]
    ident = np.eye(P, dtype=np.float32)

    g1c = np.ascontiguousarray(g1[:, None])
    b1c = np.ascontiguousarray(beta1[:, None])
    g2c = np.ascontiguousarray(g2[:, None])
    b2c = np.ascontiguousarray(beta2[:, None])

    # per-core residual slice (feature-major) and validity mask
    span = NTC * P
    xct = np.zeros((NCORES, D, span), np.float32)
    maskb = np.zeros((NCORES, P, span), np.float32)
    for c in range(NCORES):
        c0 = c * span
        v = min(max(N - c0, 0), span)
        if v > 0:
            xct[c, :, :v] = x.T[:, c0:c0 + v]
            maskb[c, :, :v] = 1.0

    in_maps = []
    for c in range(NCORES):
        in_maps.append({
            "xt": xt, "xbt": xbt,
            "wxt": wxt, "wbt": wbt, "w1t": w1t, "w2t": w2t,
            "w2tb": w2tb, "identb": identb,
            "linbb": linbb, "iotab": iotab, "ident": ident,
            "g1c": g1c, "b1c": b1c, "g2c": g2c, "b2c": b2c,
            "xct": np.ascontiguousarray(xct[c]),
            "maskb": np.ascontiguousarray(maskb[c]),
            "srcw": _pack_idx(src_pad[c].astype(np.int16)),
            "briw": _pack_idx(bri_pad[c].astype(np.int16)),
            "dstloc": np.ascontiguousarray(dloc_T[c].astype(BF16)),
        })
    meta = (tuple(int(b) for b in B), 1.0 + eps)
    return in_maps, meta


def _build(meta):
    import concourse.bacc as bacc
    import concourse.mybir as mybir
    import concourse.tile as tile

    B, resid_scale = meta
    SB = sum(B)
    S = SB * P
    f32 = mybir.dt.float32
    bf16 = mybir.dt.bfloat16
    i16 = mybir.dt.int16
    Alu = mybir.AluOpType
    Act = mybir.ActivationFunctionType
    span = NTC * P

    nc = bacc.Bacc("TRN2", target_bir_lowering=False, debug=False,
                   num_devices=NCORES, num_swdge_queues=NQ,
                   dynamic_dma_scratch_size=KSCRATCH)
    # Leave the top of SBUF for the runtime's SWDGE descriptor rings /
    # DynamicDMAScratch carveout — allocating into it wedges the device.
    nc.sbuf_top = min(nc.sbuf_top, 192 * 1024)

    def din(name, shape, dt=f32):
        return nc.dram_tensor(name, shape, dt, kind="ExternalInput")

    xt_d = din("xt", [D, N2], bf16); xbt_d = din("xbt", [D, NB2], bf16)
    wxt_d = din("wxt", [D, D], bf16); wbt_d = din("wbt", [D, D], bf16)
    w1t_d = din("w1t", [D, D])
    linbb_d = din("linbb", [P, 512]); iotab_d = din("iotab", [P, GCHUNK * P], bf16)
    xct_d = din("xct", [D, span])
    srcw_d = din("srcw", [128, S // 16], i16)
    briw_d = din("briw", [128, S // 16], i16)
    dstloc_d = din("dstloc", [P, SB], bf16)
    if KFUSE:
        w2t_d = din("w2t", [D, D])
        g1c_d = din("g1c", [P, 1]); b1c_d = din("b1c", [P, 1])
        g2c_d = din("g2c", [P, 1]); b2c_d = din("b2c", [P, 1])
        maskb_d = din("maskb", [P, span])
        ident_d = din("ident", [P, P])
        out_d = nc.dram_tensor("out", [span, D], f32, kind="ExternalOutput")
        bn_in = [nc.dram_tensor(f"bn_in{i}", [P, 2], f32) for i in range(2)]
        bn_out = [nc.dram_tensor(f"bn_out{i}", [P, 2], f32,
                                 addr_space="Shared") for i in range(2)]
    else:
        h_out_d = nc.dram_tensor("h_out", [P, span], f32, kind="ExternalOutput")
        stat_out_d = nc.dram_tensor("stat_out", [P, 2], f32,
                                    kind="ExternalOutput")

    xw_tab = nc.dram_tensor("xw_tab", [N2, D], bf16)
    ew_tab = nc.dram_tensor("ew_tab", [NB2, D], bf16)
    with tile.TileContext(nc) as tc:
        with (
            tc.tile_pool(name="consts", bufs=1) as cp,
            tc.tile_pool(name="pa_src", bufs=3) as pa_src,
            tc.tile_pool(name="pa_stg", bufs=3) as pa_stg,
            tc.tile_pool(name="psA", bufs=2, space="PSUM") as psA,
            tc.tile_pool(name="psB", bufs=2, space="PSUM") as psB,
            tc.tile_pool(name="gx", bufs=KLA + 2) as gxp,
            tc.tile_pool(name="ge", bufs=3) as gep,
            tc.tile_pool(name="oh", bufs=3) as ohp,
            tc.tile_pool(name="full", bufs=1) as fullp,
            tc.tile_pool(name="small", bufs=1) as smallp,
        ):
            def load_const(name, dram, shape, dt=f32):
                t = cp.tile(shape, dt, tag=f"c_{name}")
                nc.sync.dma_start(t[:], dram[:])
                return t

            wxt = load_const("wxt", wxt_d, [D, D], bf16)
            wbt = load_const("wbt", wbt_d, [D, D], bf16)
            w1t = load_const("w1t", w1t_d, [D, D])
            linbb = load_const("linbb", linbb_d, [P, 512])
            iotab = load_const("iotab", iotab_d, [P, GCHUNK * P], bf16)
            xct = load_const("xct", xct_d, [D, span])
            srcw = load_const("srcw", srcw_d, [128, S // 16], i16)
            briw = load_const("briw", briw_d, [128, S // 16], i16)
            dstloc = load_const("dstloc", dstloc_d, [P, SB], bf16)
            if KFUSE:
                w2t = load_const("w2t", w2t_d, [D, D])
                g1c = load_const("g1c", g1c_d, [P, 1])
                b1c = load_const("b1c", b1c_d, [P, 1])
                g2c = load_const("g2c", g2c_d, [P, 1])
                b2c = load_const("b2c", b2c_d, [P, 1])
                maskb = load_const("maskb", maskb_d, [P, span])
                ident = load_const("ident", ident_d, [P, P])

            # ---------------- Phase A: combined bf16 node table ----------------
            CW = 4096  # source columns per chunk

            def build_table(src_dram, ncols, w_sbuf, tab_dram, add_bias,
                            cw=CW):
                for c0 in range(0, ncols, cw):
                    w = min(cw, ncols - c0)
                    s = pa_src.tile([D, CW], bf16, tag="pa_src")
                    nc.sync.dma_start(s[:, :w], src_dram[:, c0:c0 + w])
                    g = pa_stg.tile([P, CW], bf16, tag="pa_stg")
                    for q0 in range(0, w, 512):
                        qw = min(512, w - q0)
                        ps = psA.tile([P, 512], f32, tag="psAb")
                        for j in range(qw // P):
                            nc.tensor.matmul(
                                ps[:, j * P:(j + 1) * P],
                                s[:, q0 + j * P:q0 + (j + 1) * P], w_sbuf[:])
                        sl = g[:, q0:q0 + qw]
                        if add_bias:
                            nc.vector.tensor_tensor(sl, ps[:, :qw],
                                                    linbb[:, :qw], Alu.add)
                        else:
                            nc.scalar.activation(sl, ps[:, :qw], Act.Copy)
                    nc.scalar.dma_start(
                        tab_dram[c0:c0 + w, :].rearrange(
                            "(g p) d -> p g d", p=P),
                        g[:, :w].rearrange("p (g d) -> p g d", d=P))

            # ---------------- Phase B: gather + scatter-add ----------------
            # chunk list: (lt, batch_off, cb, first_of_tile, last_of_tile)
            chunks = []
            gbase = 0
            for lt in range(NTC):
                nb = B[lt]
                done = 0
                while done < nb:
                    cb = min(GCHUNK, nb - done)
                    chunks.append((lt, gbase + done, cb, done == 0,
                                   done + cb == nb))
                    done += cb
                gbase += nb

            outT = fullp.tile([P, span], f32, tag="outT")
            h1 = fullp.tile([P, span], f32, tag="h")
            s_parts = smallp.tile([P, 2 * NTC], f32, tag="s_parts")
            gq = [0]
            gx_tiles = {}

            def emit_gx(j):
                lt, boff, cb, _, _ = chunks[j]
                ne = cb * P
                gx = gxp.tile([P, GCHUNK, D], bf16, tag="gx")
                nc.gpsimd.dma_gather(
                    gx[:, :cb, :], xw_tab[:],
                    srcw[:, boff * 8:boff * 8 + ne // 16],
                    ne, ne, D, single_packet=False, queue_num=gq[0] % NQ)
                gq[0] += 1
                gx_tiles[j] = gx

            # x-table, then early x-gathers overlap the e-table build
            build_table(xt_d, N2, wxt, xw_tab, False)
            LA = min(KLA, len(chunks))
            for j in range(LA):
                emit_gx(j)
            build_table(xbt_d, NB2, wbt, ew_tab, True)

            aggT = None
            for j, (lt, boff, cb, first_c, last_c) in enumerate(chunks):
                ne = cb * P
                if first_c:
                    aggT = psB.tile([P, P], f32, tag="aggT")
                ge = gep.tile([P, GCHUNK, D], bf16, tag="ge")
                nc.gpsimd.dma_gather(
                    ge[:, :cb, :], ew_tab[:],
                    briw[:, boff * 8:boff * 8 + ne // 16],
                    ne, ne, D, single_packet=False, queue_num=gq[0] % NQ)
                gq[0] += 1
                if j + LA < len(chunks):
                    emit_gx(j + LA)
                gx = gx_tiles.pop(j)

                nc.vector.tensor_tensor(gx[:, :cb, :], gx[:, :cb, :],
                                        ge[:, :cb, :], Alu.add)
                nc.scalar.activation(gx[:, :cb, :], gx[:, :cb, :], Act.Relu)

                oh = ohp.tile([P, GCHUNK * P], bf16, tag="oh")
                dl = dstloc[:, boff:boff + cb]
                dl_rep = dl.to_broadcast((P, cb, P))
                nc.vector.tensor_tensor(
                    oh[:, :cb * P].rearrange("p (b c) -> p b c", c=P),
                    iotab[:, :cb * P].rearrange("p (b c) -> p b c", c=P),
                    dl_rep, Alu.is_equal)
                for b in range(cb):
                    nc.tensor.matmul(
                        aggT[:], gx[:, b, :], oh[:, b * P:(b + 1) * P],
                        start=(first_c and b == 0), stop=(last_c and b == cb - 1))

                if last_c:
                    # residual + linear1 + stats + h1-out for this tile,
                    # all under later tiles' gathers
                    sl = slice(lt * P, (lt + 1) * P)
                    nc.vector.scalar_tensor_tensor(
                        outT[:, sl], xct[:, sl], float(resid_scale), aggT[:],
                        Alu.mult, Alu.add)
                    ps2 = psA.tile([P, P], f32, tag="psA")
                    nc.tensor.matmul(ps2[:], w1t[:], outT[:, sl])
                    nc.scalar.activation(h1[:, sl], ps2[:], Act.Copy)
                    nc.vector.reduce_sum(s_parts[:, 2 * lt:2 * lt + 1],
                                         h1[:, sl], axis=mybir.AxisListType.X)
                    sqt = smallp.tile([P, P], f32, tag="sqt")
                    nc.vector.tensor_tensor(sqt[:], h1[:, sl], h1[:, sl],
                                            Alu.mult)
                    nc.vector.reduce_sum(s_parts[:, 2 * lt + 1:2 * lt + 2],
                                         sqt[:], axis=mybir.AxisListType.X)
                    nc.sync.dma_start(h_out_d[:, sl], h1[:, sl])

            # ---------------- Phase C head: per-core BN stats --------
            s_stat = smallp.tile([P, 2], f32, tag="stat1")
            nc.vector.reduce_sum(
                s_stat[:, 0:1],
                s_parts[:].rearrange("p (t two) -> p t two", two=2)[:, :, 0:1],
                axis=mybir.AxisListType.XY)
            nc.vector.reduce_sum(
                s_stat[:, 1:2],
                s_parts[:].rearrange("p (t two) -> p t two", two=2)[:, :, 1:2],
                axis=mybir.AxisListType.XY)
            if not KFUSE:
                nc.sync.dma_start(stat_out_d[:], s_stat[:])
            else:
                # ---- fused tail: BN1 -> relu -> mask -> linear2 -> BN2 ----
                def allreduce_stats(idx, stat_tile):
                    nc.sync.dma_start(bn_in[idx][:], stat_tile[:])
                    nc.gpsimd.collective_compute(
                        "AllReduce", Alu.add,
                        replica_groups=[list(range(NCORES))],
                        ins=[bn_in[idx][:]], outs=[bn_out[idx][:]])
                    red = smallp.tile([P, 2], f32, tag=f"red{idx}")
                    nc.sync.dma_start(red[:], bn_out[idx][:])
                    return red

                red0 = allreduce_stats(0, s_stat)
                a0, sh0 = _bn_scale_shift(nc, mybir, smallp, red0, g1c, b1c, 0)
                # BN1-relu in place on h1, then mask
                nc.scalar.activation(h1[:], h1[:], Act.Relu,
                                     bias=sh0[:], scale=a0[:])
                nc.vector.tensor_tensor(h1[:], h1[:], maskb[:], Alu.mult)

                h2 = fullp.tile([P, span], f32, tag="h2")
                for lt in range(NTC):
                    sl = slice(lt * P, (lt + 1) * P)
                    ps = psA.tile([P, P], f32, tag="psA")
                    nc.tensor.matmul(ps[:], w2t[:], h1[:, sl])
                    nc.scalar.activation(h2[:, sl], ps[:], Act.Copy)
                s2 = smallp.tile([P, 2], f32, tag="stat2")
                nc.vector.reduce_sum(s2[:, 0:1], h2[:],
                                     axis=mybir.AxisListType.X)
                sq2 = fullp.tile([P, span], f32, tag="sq")
                nc.vector.tensor_tensor(sq2[:], h2[:], h2[:], Alu.mult)
                nc.vector.reduce_sum(s2[:, 1:2], sq2[:],
                                     axis=mybir.AxisListType.X)
                red1 = allreduce_stats(1, s2)
                a1, sh1 = _bn_scale_shift(nc, mybir, smallp, red1, g2c, b2c, 1)
                nc.scalar.activation(h2[:], h2[:], Act.Relu,
                                     bias=sh1[:], scale=a1[:])

                stg = fullp.tile([P, span], f32, tag="sq")
                for lt in range(NTC):
                    sl = slice(lt * P, (lt + 1) * P)
                    ps = psA.tile([P, P], f32, tag="psA")
                    nc.tensor.transpose(ps[:], h2[:, sl], ident[:])
                    nc.scalar.activation(stg[:, sl], ps[:], Act.Copy)
                nc.sync.dma_start(
                    out_d[:].rearrange("(g p) d -> p g d", p=P),
                    stg[:].rearrange("p (g d) -> p g d", d=P))

    nc.compile()
    return nc


_cache2 = {}


def _bn_scale_shift(nc, mybir, smallp, red, gcol, bcol, idx):
    """Device-side BN coefficients from reduced stats: a = g*rstd, sh = b - mu*a."""
    f32 = mybir.dt.float32
    Alu = mybir.AluOpType
    Act = mybir.ActivationFunctionType
    mu = smallp.tile([P, 1], f32, tag=f"mu{idx}")
    nc.vector.tensor_scalar(mu[:], red[:, 0:1], 1.0 / N, None, Alu.mult)
    mu2 = smallp.tile([P, 1], f32, tag=f"mu2{idx}")
    nc.vector.tensor_tensor(mu2[:], mu[:], mu[:], Alu.mult)
    e2 = smallp.tile([P, 1], f32, tag=f"e2{idx}")
    nc.vector.tensor_scalar(e2[:], red[:, 1:2], 1.0 / N, None, Alu.mult)
    var = smallp.tile([P, 1], f32, tag=f"var{idx}")
    nc.vector.tensor_tensor(var[:], e2[:], mu2[:], Alu.subtract)
    vep = smallp.tile([P, 1], f32, tag=f"vep{idx}")
    nc.vector.tensor_scalar(vep[:], var[:], BN_EPS, None, Alu.add)
    std = smallp.tile([P, 1], f32, tag=f"std{idx}")
    nc.scalar.activation(std[:], vep[:], Act.Sqrt)
    rstd = smallp.tile([P, 1], f32, tag=f"rstd{idx}")
    nc.vector.reciprocal(rstd[:], std[:])
    a = smallp.tile([P, 1], f32, tag=f"a{idx}")
    nc.vector.tensor_tensor(a[:], gcol[:], rstd[:], Alu.mult)
    tmp = smallp.tile([P, 1], f32, tag=f"tmp{idx}")
    nc.vector.tensor_tensor(tmp[:], mu[:], a[:], Alu.mult)
    sh = smallp.tile([P, 1], f32, tag=f"sh{idx}")
    nc.vector.tensor_tensor(sh[:], bcol[:], tmp[:], Alu.subtract)
    return a, sh


def _build_phase2():
    """NEFF2: h1n = mask*relu(BN1(h1)); h2 = h1n @ w2.T; per-core stats of h2."""
    import concourse.bacc as bacc
    import concourse.mybir as mybir
    import concourse.tile as tile

    f32 = mybir.dt.float32
    Alu = mybir.AluOpType
    Act = mybir.ActivationFunctionType
    span = NTC * P

    nc = bacc.Bacc("TRN2", target_bir_lowering=False, debug=False,
                   num_devices=NCORES)
    nc.sbuf_top = min(nc.sbuf_top, 192 * 1024)

    def din(name, shape):
        return nc.dram_tensor(name, shape, f32, kind="ExternalInput")

    h_d = din("h_in", [P, span])
    red_d = din("red", [P, 2])
    w2t_d = din("w2t", [D, D])
    g1c_d = din("g1c", [P, 1]); b1c_d = din("b1c", [P, 1])
    maskb_d = din("maskb", [P, span])
    h_out_d = nc.dram_tensor("h_out", [P, span], f32, kind="ExternalOutput")
    stat_out_d = nc.dram_tensor("stat_out", [P, 2], f32, kind="ExternalOutput")

    with tile.TileContext(nc) as tc:
        with (
            tc.tile_pool(name="consts", bufs=1) as cp,
            tc.tile_pool(name="psA", bufs=2, space="PSUM") as psA,
            tc.tile_pool(name="full", bufs=1) as fullp,
            tc.tile_pool(name="small", bufs=1) as smallp,
        ):
            def load_const(name, dram, shape):
                t = cp.tile(shape, f32, tag=f"c_{name}")
                nc.sync.dma_start(t[:], dram[:])
                return t

            h1 = load_const("h", h_d, [P, span])
            red = load_const("red", red_d, [P, 2])
            w2t = load_const("w2t", w2t_d, [D, D])
            g1c = load_const("g1c", g1c_d, [P, 1])
            b1c = load_const("b1c", b1c_d, [P, 1])
            maskb = load_const("maskb", maskb_d, [P, span])

            a, sh = _bn_scale_shift(nc, mybir, smallp, red, g1c, b1c, 0)
            hn = fullp.tile([P, span], f32, tag="hn")
            nc.scalar.activation(hn[:], h1[:], Act.Relu, bias=sh[:], scale=a[:])
            nc.vector.tensor_tensor(hn[:], hn[:], maskb[:], Alu.mult)

            h2 = fullp.tile([P, span], f32, tag="h2")
            for lt in range(NTC):
                sl = slice(lt * P, (lt + 1) * P)
                ps = psA.tile([P, P], f32, tag="psA")
                nc.tensor.matmul(ps[:], w2t[:], hn[:, sl])
                nc.scalar.activation(h2[:, sl], ps[:], Act.Copy)
            s_stat = smallp.tile([P, 2], f32, tag="stat2")
            nc.vector.reduce_sum(s_stat[:, 0:1], h2[:],
                                 axis=mybir.AxisListType.X)
            sq = fullp.tile([P, span], f32, tag="sq")
            nc.vector.tensor_tensor(sq[:], h2[:], h2[:], Alu.mult)
            nc.vector.reduce_sum(s_stat[:, 1:2], sq[:],
                                 axis=mybir.AxisListType.X)
            nc.sync.dma_start(stat_out_d[:], s_stat[:])
            nc.sync.dma_start(h_out_d[:], h2[:])

    nc.compile()
    return nc


def _build_phase3():
    """NEFF3: out = transpose(relu(BN2(h2)))."""
    import concourse.bacc as bacc
    import concourse.mybir as mybir
    import concourse.tile as tile

    f32 = mybir.dt.float32
    Act = mybir.ActivationFunctionType
    span = NTC * P

    nc = bacc.Bacc("TRN2", target_bir_lowering=False, debug=False,
                   num_devices=NCORES)
    nc.sbuf_top = min(nc.sbuf_top, 192 * 1024)

    def din(name, shape):
        return nc.dram_tensor(name, shape, f32, kind="ExternalInput")

    h_d = din("h_in", [P, span])
    red_d = din("red", [P, 2])
    g2c_d = din("g2c", [P, 1]); b2c_d = din("b2c", [P, 1])
    ident_d = din("ident", [P, P])
    out_d = nc.dram_tensor("out", [span, D], f32, kind="ExternalOutput")

    with tile.TileContext(nc) as tc:
        with (
            tc.tile_pool(name="consts", bufs=1) as cp,
            tc.tile_pool(name="psA", bufs=2, space="PSUM") as psA,
            tc.tile_pool(name="full", bufs=1) as fullp,
            tc.tile_pool(name="small", bufs=1) as smallp,
        ):
            def load_const(name, dram, shape):
                t = cp.tile(shape, f32, tag=f"c_{name}")
                nc.sync.dma_start(t[:], dram[:])
                return t

            h2 = load_const("h", h_d, [P, span])
            red = load_const("red", red_d, [P, 2])
            g2c = load_const("g2c", g2c_d, [P, 1])
            b2c = load_const("b2c", b2c_d, [P, 1])
            ident = load_const("ident", ident_d, [P, P])

            a, sh = _bn_scale_shift(nc, mybir, smallp, red, g2c, b2c, 1)
            hn = fullp.tile([P, span], f32, tag="hn")
            nc.scalar.activation(hn[:], h2[:], Act.Relu, bias=sh[:], scale=a[:])

            stg = fullp.tile([P, span], f32, tag="stg")
            for lt in range(NTC):
                sl = slice(lt * P, (lt + 1) * P)
                ps = psA.tile([P, P], f32, tag="psA")
                nc.tensor.transpose(ps[:], hn[:, sl], ident[:])
                nc.scalar.activation(stg[:, sl], ps[:], Act.Copy)
            nc.sync.dma_start(
                out_d[:].rearrange("(g p) d -> p g d", p=P),
                stg[:].rearrange("p (g d) -> p g d", d=P))

    nc.compile()
    return nc


def _build_phase23():
    """Single second NEFF: every core gets the full (rotated) bf16 h1 with
    its own span at columns [0:span] and exact zeros appended for the 240
    padded nodes. BN1-apply + linear2 + global BN2 stats are computed
    replicated (mask-free, exact); BN2-apply + transpose + output only for
    the own span. Removes one NEFF launch and one host sync."""
    import concourse.bacc as bacc
    import concourse.mybir as mybir
    import concourse.tile as tile

    f32 = mybir.dt.float32
    bf16 = mybir.dt.bfloat16
    Alu = mybir.AluOpType
    Act = mybir.ActivationFunctionType
    span = NTC * P
    FULL = N2  # 10240 = 10000 real + 240 zero columns

    nc = bacc.Bacc("TRN2", target_bir_lowering=False, debug=False,
                   num_devices=NCORES)
    nc.sbuf_top = min(nc.sbuf_top, 192 * 1024)

    h_d = nc.dram_tensor("h_all", [P, FULL], bf16, kind="ExternalInput")
    red_d = nc.dram_tensor("red", [P, 2], f32, kind="ExternalInput")
    w2tb_d = nc.dram_tensor("w2tb", [D, D], bf16, kind="ExternalInput")
    g1c_d = nc.dram_tensor("g1c", [P, 1], f32, kind="ExternalInput")
    b1c_d = nc.dram_tensor("b1c", [P, 1], f32, kind="ExternalInput")
    g2c_d = nc.dram_tensor("g2c", [P, 1], f32, kind="ExternalInput")
    b2c_d = nc.dram_tensor("b2c", [P, 1], f32, kind="ExternalInput")
    ident_d = nc.dram_tensor("identb", [P, P], bf16, kind="ExternalInput")
    out_d = nc.dram_tensor("out", [span, D], f32, kind="ExternalOutput")

    with tile.TileContext(nc) as tc:
        with (
            tc.tile_pool(name="consts", bufs=1) as cp,
            tc.tile_pool(name="ps", bufs=2, space="PSUM") as psp,
            tc.tile_pool(name="full", bufs=1) as fullp,
            tc.tile_pool(name="small", bufs=1) as smallp,
        ):
            def load_const(name, dram, shape, dt=f32):
                t = cp.tile(shape, dt, tag=f"c_{name}")
                nc.sync.dma_start(t[:], dram[:])
                return t

            h1 = load_const("h", h_d, [P, FULL], bf16)
            red = load_const("red", red_d, [P, 2])
            w2tb = load_const("w2tb", w2tb_d, [D, D], bf16)
            g1c = load_const("g1c", g1c_d, [P, 1])
            b1c = load_const("b1c", b1c_d, [P, 1])
            g2c = load_const("g2c", g2c_d, [P, 1])
            b2c = load_const("b2c", b2c_d, [P, 1])
            ident = load_const("ident", ident_d, [P, P], bf16)

            a0, sh0 = _bn_scale_shift(nc, mybir, smallp, red, g1c, b1c, 0)
            # BN1-apply + relu in place (bf16); zero columns stay relu(sh0)
            # only in the 240 appended pad columns... they must be zeroed
            # for exact stats: recompute them as zeros via memset after.
            nc.scalar.activation(h1[:], h1[:], Act.Relu,
                                 bias=sh0[:], scale=a0[:])
            nc.vector.memset(h1[:, N:FULL], 0.0)

            # linear2 over the full width, bf16, w2 stationary
            h2 = fullp.tile([P, FULL], bf16, tag="h2")
            for q0 in range(0, FULL, 512):
                ps = psp.tile([P, 512], f32, tag="ps")
                for j in range(4):
                    nc.tensor.matmul(ps[:, j * P:(j + 1) * P], w2tb[:],
                                     h1[:, q0 + j * P:q0 + (j + 1) * P])
                nc.scalar.activation(h2[:, q0:q0 + 512], ps[:], Act.Copy)

            # global BN2 stats (exact: pad columns of h2 = W2 @ 0 = 0)
            s2 = smallp.tile([P, 2], f32, tag="s2")
            nc.vector.reduce_sum(s2[:, 0:1], h2[:], axis=mybir.AxisListType.X)
            sq = fullp.tile([P, FULL], f32, tag="sq")
            nc.vector.tensor_tensor(sq[:], h2[:], h2[:], Alu.mult)
            nc.vector.reduce_sum(s2[:, 1:2], sq[:], axis=mybir.AxisListType.X)
            a1, sh1 = _bn_scale_shift(nc, mybir, smallp, s2, g2c, b2c, 1)

            # BN2-apply + relu + transpose for the own span only
            hn2 = fullp.tile([P, span], bf16, tag="hn2")
            nc.scalar.activation(hn2[:], h2[:, :span], Act.Relu,
                                 bias=sh1[:], scale=a1[:])
            stg = fullp.tile([P, span], f32, tag="stg")
            for lt in range(NTC):
                sl = slice(lt * P, (lt + 1) * P)
                ps = psp.tile([P, P], bf16, tag="pst")
                nc.tensor.transpose(ps[:], hn2[:, sl], ident[:])
                nc.scalar.activation(stg[:, sl], ps[:], Act.Copy)
            nc.sync.dma_start(
                out_d[:].rearrange("(g p) d -> p g d", p=P),
                stg[:].rearrange("p (g d) -> p g d", d=P))

    nc.compile()
    return nc


def kernel(**inputs):
    global last_results
    from concourse.bass_utils import run_bass_kernel_spmd

    in_maps, meta = _host_prep(inputs)
    if meta not in _cache:
        _cache[meta] = _build(meta)
    if not KFUSE and KP23 and "p23" not in _cache2:
        _cache2["p23"] = _build_phase23()
    if not KFUSE and not KP23 and "p2" not in _cache2:
        _cache2["p2"] = _build_phase2()
        _cache2["p3"] = _build_phase3()
    nc1 = _cache[meta]
    nc2 = _cache2.get("p2"); nc3 = _cache2.get("p3")
    cores = list(range(NCORES))
    trace = bool(os.environ.get("KERNEL_TRACE"))

    n1_keys = ["xt", "xbt", "wxt", "wbt", "w1t", "linbb", "iotab",
               "xct", "srcw", "briw", "dstloc"]
    if KFUSE:
        n1_keys += ["w2t", "g1c", "b1c", "g2c", "b2c", "maskb", "ident"]
        in1 = [{k: in_maps[c][k] for k in n1_keys} for c in range(NCORES)]
        res1 = run_bass_kernel_spmd(nc1, in1, cores, trace=trace)
        last_results = (res1,)
        out = np.concatenate([res1.results[c]["out"] for c in range(NCORES)],
                             axis=0)
        return np.ascontiguousarray(out[:N])
    in1 = [{k: in_maps[c][k] for k in n1_keys} for c in range(NCORES)]
    res1 = run_bass_kernel_spmd(nc1, in1, cores, trace=trace)
    red1 = np.sum([res1.results[c]["stat_out"] for c in range(NCORES)], axis=0)
    if KP23:
        span = NTC * P
        hb = np.concatenate([res1.results[c]["h_out"] for c in range(NCORES)],
                            axis=1).astype(BF16)  # [P, 10240]
        zpad = np.zeros((P, N2 - N), BF16)
        in2 = []
        for c in range(NCORES):
            roll = np.concatenate(
                [hb[:, span * c:N], hb[:, :span * c], zpad], axis=1)
            in2.append({"h_all": np.ascontiguousarray(roll), "red": red1,
                        "w2tb": in_maps[c]["w2tb"],
                        "identb": in_maps[c]["identb"],
                        "g1c": in_maps[c]["g1c"], "b1c": in_maps[c]["b1c"],
                        "g2c": in_maps[c]["g2c"], "b2c": in_maps[c]["b2c"]})
        res2 = run_bass_kernel_spmd(_cache2["p23"], in2, cores, trace=trace)
        last_results = (res1, res2)
        out = np.concatenate([res2.results[c]["out"] for c in range(NCORES)],
                             axis=0)
        return np.ascontiguousarray(out[:N])
    in2 = [{"h_in": res1.results[c]["h_out"], "red": red1,
            "w2t": in_maps[c]["w2t"], "g1c": in_maps[c]["g1c"],
            "b1c": in_maps[c]["b1c"], "maskb": in_maps[c]["maskb"]}
           for c in range(NCORES)]
    res2 = run_bass_kernel_spmd(nc2, in2, cores, trace=trace)
    red2 = np.sum([res2.results[c]["stat_out"] for c in range(NCORES)], axis=0)
    in3 = [{"h_in": res2.results[c]["h_out"], "red": red2,
            "g2c": in_maps[c]["g2c"], "b2c": in_maps[c]["b2c"],
            "ident": in_maps[c]["ident"]} for c in range(NCORES)]
    res3 = run_bass_kernel_spmd(nc3, in3, cores, trace=trace)

    last_results = (res1, res2, res3)
    out = np.concatenate([res3.results[c]["out"] for c in range(NCORES)], axis=0)
    return np.ascontiguousarray(out[:N])


# revision 30
# speedup vs baseline: 1.0638x; 1.0066x over previous
"""Trainium2 Bass kernel for nn_AdjacencyConv (GNN message passing).

Reference computation:
    msg  = relu(concat[x[src], x_bridge[bri]] @ lin_w.T + lin_b)   # [E, D]
    agg  = segment_sum(msg, dst, N)                                # [N, D]
    out  = agg + (1+eps)*x
    h    = relu(BN(out @ w1.T + b1)); h = relu(BN(h @ w2.T + b2))  # train-mode BN

Device algorithm (8-core SPMD, edges sharded by dst node-tile):
  Phase A (per core, replicated): build bf16 node tables in DRAM
        xw_tab = x @ Wx.T            (Wx = lin_w[:, :D])
        ew_tab = x_bridge @ Wb.T + b (Wb = lin_w[:, D:])
    so the per-edge linear factorizes: msg = relu(xw_tab[src] + ew_tab[bri]).
    The x-table is built first; the first KLA chunks' x-gathers are emitted
    before the e-table build so SWDGE descriptor generation (the kernel's
    bottleneck, ~3ns/idx serial on the GpSimd engine) overlaps phase A.
  Phase B: per dst node-tile of 128 nodes, per chunk of GCHUNK 128-edge
    batches: dma_gather x-rows and e-rows (bf16, 256B descriptors,
    4 SWDGE queues, x-gathers KLA chunks ahead) edge-major [128e, cb, D],
    add + relu in bf16, scatter-add via bf16 one-hot matmuls accumulating
    feature-major agg in PSUM (f32). Residual, linear1, per-tile BN stats
    and the h1 output DMA all run per-tile under later tiles' gathers.
  Phase C: BN stats are reduced across cores host-side between NEFFs
    (biases b1/b2 cancel in BN and are dropped; on-device AllReduce was
    measured ~170us per collective here — host reduction is faster).
    NEFF2 = BN1+relu+linear2, NEFF3 = BN2+relu+transpose to node-major.

Host side does only layout preprocessing: index sorting/padding/packing,
dtype casts, weight transposes, and output assembly.
"""

import os
import numpy as np
import ml_dtypes

BF16 = ml_dtypes.bfloat16

N, NB, E, D = 10000, 20000, 640000, 128
P = 128
NCORES = 8
NTC = 10                    # node tiles per core
N2 = NCORES * NTC * P       # 10240 padded nodes
NB2 = 20096                 # x_bridge padded to multiple of 128
R2 = N2 + NB2               # combined table rows
BN_EPS = 1e-5
GCHUNK = int(os.environ.get("KGCHUNK", "16"))  # batches per dma_gather
NQ = int(os.environ.get("KNQ", "4"))           # SWDGE queues
KSCRATCH = int(os.environ.get("KSCRATCH", "16384"))
KGB = int(os.environ.get("KGB", "3"))          # gather pool bufs
KFUSE = int(os.environ.get("KFUSE", "0"))      # 1 = single fused NEFF
KLA = int(os.environ.get("KLA", "10"))          # x-gather lookahead chunks
KP23 = int(os.environ.get("KP23", "0"))        # merged phase-2/3 NEFF

_cache = {}

# exposed for test.py
last_results = None


def _pack_idx(idx16):
    """Wrap an int16 index vector for dma_gather: [16, n/16] replicated x8."""
    w = idx16.reshape(-1, 16).T
    return np.tile(w, (8, 1)).copy()


def _host_prep(inputs):
    x = np.asarray(inputs["x"], np.float32)
    xb = np.asarray(inputs["x_bridge"], np.float32)
    ei = np.asarray(inputs["edge_index"])
    bri = np.asarray(inputs["bridge_index"])
    lin_w = np.asarray(inputs["lin_w"], np.float32)
    lin_b = np.asarray(inputs["lin_b"], np.float32)
    eps = float(np.asarray(inputs["eps"]).reshape(-1)[0])
    w1 = np.asarray(inputs["w1"], np.float32)
    g1 = np.asarray(inputs["g1"], np.float32)
    beta1 = np.asarray(inputs["beta1"], np.float32)
    w2 = np.asarray(inputs["w2"], np.float32)
    g2 = np.asarray(inputs["g2"], np.float32)
    beta2 = np.asarray(inputs["beta2"], np.float32)

    src = ei[0].astype(np.int64)
    dst = ei[1].astype(np.int64)
    bri = bri.astype(np.int64)

    # sort edges by dst, bucket into 128-node tiles
    order = np.argsort(dst, kind="stable")
    dsts = dst[order]
    srcs = src[order]
    bris = bri[order]
    gt_bounds = np.searchsorted(dsts, np.arange(NCORES * NTC + 1) * P)

    # uniform program structure: per local tile lt, same batch count across cores
    cnt = np.diff(gt_bounds)  # edges per global tile (len 80)
    cnt = cnt.reshape(NCORES, NTC)
    ceils = -(-cnt // P)
    B = np.maximum(1, ceils.max(axis=0))  # batches per local tile (len NTC)
    SB = int(B.sum())          # total batches per core
    S = SB * P                 # padded edges per core

    src_pad = np.zeros((NCORES, S), np.int64)
    bri_pad = np.zeros((NCORES, S), np.int64)
    dloc_pad = np.full((NCORES, S), 999.0, np.float32)
    for c in range(NCORES):
        off = 0
        for lt in range(NTC):
            gt = c * NTC + lt
            a, b = gt_bounds[gt], gt_bounds[gt + 1]
            n = b - a
            src_pad[c, off:off + n] = srcs[a:b]
            bri_pad[c, off:off + n] = bris[a:b]
            dloc_pad[c, off:off + n] = (dsts[a:b] - gt * P).astype(np.float32)
            off += B[lt] * P

    # dstloc transposed: [128, SB], column j = batch j's 128 local-dst values
    dloc_T = np.ascontiguousarray(
        dloc_pad.reshape(NCORES, SB, P).transpose(0, 2, 1))

    # feature-major bf16 padded inputs for the table build
    xt = np.zeros((D, N2), BF16)
    xt[:, :N] = x.T.astype(BF16)
    xbt = np.zeros((D, NB2), BF16)
    xbt[:, :NB] = xb.T.astype(BF16)

    wxt = np.ascontiguousarray(lin_w[:, :D].T).astype(BF16)   # [in_f, out]
    wbt = np.ascontiguousarray(lin_w[:, D:].T).astype(BF16)
    w1t = np.ascontiguousarray(w1.T)
    w2t = np.ascontiguousarray(w2.T)
    w2tb = w2t.astype(BF16)
    identb = np.eye(P, dtype=BF16)
    linbb = np.tile(lin_b[None, :], (P, 4)).astype(np.float32)   # [128, 512]
    iotab = np.tile(np.arange(P, dtype=np.float32)[None, :],
                    (P, GCHUNK)).astype(BF16)  # [128, GCHUNK*128]
    ident = np.eye(P, dtype=np.float32)

    g1c = np.ascontiguousarray(g1[:, None])
    b1c = np.ascontiguousarray(beta1[:, None])
    g2c = np.ascontiguousarray(g2[:, None])
    b2c = np.ascontiguousarray(beta2[:, None])

    # per-core residual slice (feature-major) and validity mask
    span = NTC * P
    xct = np.zeros((NCORES, D, span), np.float32)
    maskb = np.zeros((NCORES, P, span), np.float32)
    for c in range(NCORES):
        c0 = c * span
        v = min(max(N - c0, 0), span)
        if v > 0:
            xct[c, :, :v] = x.T[:, c0:c0 + v]
            maskb[c, :, :v] = 1.0

    in_maps = []
    for c in range(NCORES):
        in_maps.append({
            "xt": xt, "xbt": xbt,
            "wxt": wxt, "wbt": wbt, "w1t": w1t, "w2t": w2t,
            "w2tb": w2tb, "identb": identb,
            "linbb": linbb, "iotab": iotab, "ident": ident,
            "g1c": g1c, "b1c": b1c, "g2c": g2c, "b2c": b2c,
            "xct": np.ascontiguousarray(xct[c]),
            "maskb": np.ascontiguousarray(maskb[c]),
            "srcw": _pack_idx(src_pad[c].astype(np.int16)),
            "briw": _pack_idx(bri_pad[c].astype(np.int16)),
            "dstloc": np.ascontiguousarray(dloc_T[c].astype(BF16)),
        })
    meta = (tuple(int(b) for b in B), 1.0 + eps)
    return in_maps, meta


def _build(meta):
    import concourse.bacc as bacc
    import concourse.mybir as mybir
    import concourse.tile as tile

    B, resid_scale = meta
    SB = sum(B)
    S = SB * P
    f32 = mybir.dt.float32
    bf16 = mybir.dt.bfloat16
    i16 = mybir.dt.int16
    Alu = mybir.AluOpType
    Act = mybir.ActivationFunctionType
    span = NTC * P

    nc = bacc.Bacc("TRN2", target_bir_lowering=False, debug=False,
                   num_devices=NCORES, num_swdge_queues=NQ,
                   dynamic_dma_scratch_size=KSCRATCH)
    # Leave the top of SBUF for the runtime's SWDGE descriptor rings /
    # DynamicDMAScratch carveout — allocating into it wedges the device.
    nc.sbuf_top = min(nc.sbuf_top, 192 * 1024)

    def din(name, shape, dt=f32):
        return nc.dram_tensor(name, shape, dt, kind="ExternalInput")

    xt_d = din("xt", [D, N2], bf16); xbt_d = din("xbt", [D, NB2], bf16)
    wxt_d = din("wxt", [D, D], bf16); wbt_d = din("wbt", [D, D], bf16)
    w1t_d = din("w1t", [D, D])
    linbb_d = din("linbb", [P, 512]); iotab_d = din("iotab", [P, GCHUNK * P], bf16)
    xct_d = din("xct", [D, span])
    srcw_d = din("srcw", [128, S // 16], i16)
    briw_d = din("briw", [128, S // 16], i16)
    dstloc_d = din("dstloc", [P, SB], bf16)
    if KFUSE:
        w2t_d = din("w2t", [D, D])
        g1c_d = din("g1c", [P, 1]); b1c_d = din("b1c", [P, 1])
        g2c_d = din("g2c", [P, 1]); b2c_d = din("b2c", [P, 1])
        maskb_d = din("maskb", [P, span])
        ident_d = din("ident", [P, P])
        out_d = nc.dram_tensor("out", [span, D], f32, kind="ExternalOutput")
        bn_in = [nc.dram_tensor(f"bn_in{i}", [P, 2], f32) for i in range(2)]
        bn_out = [nc.dram_tensor(f"bn_out{i}", [P, 2], f32,
                                 addr_space="Shared") for i in range(2)]
    else:
        h_out_d = nc.dram_tensor("h_out", [P, span], f32, kind="ExternalOutput")
        stat_out_d = nc.dram_tensor("stat_out", [P, 2], f32,
                                    kind="ExternalOutput")

    xw_tab = nc.dram_tensor("xw_tab", [N2, D], bf16)
    ew_tab = nc.dram_tensor("ew_tab", [NB2, D], bf16)
    with tile.TileContext(nc) as tc:
        with (
            tc.tile_pool(name="consts", bufs=1) as cp,
            tc.tile_pool(name="pa_src", bufs=3) as pa_src,
            tc.tile_pool(name="pa_stg", bufs=3) as pa_stg,
            tc.tile_pool(name="psA", bufs=2, space="PSUM") as psA,
            tc.tile_pool(name="psB", bufs=2, space="PSUM") as psB,
            tc.tile_pool(name="gx", bufs=KLA + 2) as gxp,
            tc.tile_pool(name="ge", bufs=3) as gep,
            tc.tile_pool(name="oh", bufs=3) as ohp,
            tc.tile_pool(name="full", bufs=1) as fullp,
            tc.tile_pool(name="small", bufs=1) as smallp,
        ):
            def load_const(name, dram, shape, dt=f32):
                t = cp.tile(shape, dt, tag=f"c_{name}")
                nc.sync.dma_start(t[:], dram[:])
                return t

            wxt = load_const("wxt", wxt_d, [D, D], bf16)
            wbt = load_const("wbt", wbt_d, [D, D], bf16)
            w1t = load_const("w1t", w1t_d, [D, D])
            linbb = load_const("linbb", linbb_d, [P, 512])
            iotab = load_const("iotab", iotab_d, [P, GCHUNK * P], bf16)
            xct = load_const("xct", xct_d, [D, span])
            srcw = load_const("srcw", srcw_d, [128, S // 16], i16)
            briw = load_const("briw", briw_d, [128, S // 16], i16)
            dstloc = load_const("dstloc", dstloc_d, [P, SB], bf16)
            if KFUSE:
                w2t = load_const("w2t", w2t_d, [D, D])
                g1c = load_const("g1c", g1c_d, [P, 1])
                b1c = load_const("b1c", b1c_d, [P, 1])
                g2c = load_const("g2c", g2c_d, [P, 1])
                b2c = load_const("b2c", b2c_d, [P, 1])
                maskb = load_const("maskb", maskb_d, [P, span])
                ident = load_const("ident", ident_d, [P, P])

            # ---------------- Phase A: combined bf16 node table ----------------
            CW = 4096  # source columns per chunk

            def build_table(src_dram, ncols, w_sbuf, tab_dram, add_bias,
                            cw=CW):
                for c0 in range(0, ncols, cw):
                    w = min(cw, ncols - c0)
                    s = pa_src.tile([D, CW], bf16, tag="pa_src")
                    nc.sync.dma_start(s[:, :w], src_dram[:, c0:c0 + w])
                    g = pa_stg.tile([P, CW], bf16, tag="pa_stg")
                    for q0 in range(0, w, 512):
                        qw = min(512, w - q0)
                        ps = psA.tile([P, 512], f32, tag="psAb")
                        for j in range(qw // P):
                            nc.tensor.matmul(
                                ps[:, j * P:(j + 1) * P],
                                s[:, q0 + j * P:q0 + (j + 1) * P], w_sbuf[:])
                        sl = g[:, q0:q0 + qw]
                        if add_bias:
                            nc.vector.tensor_tensor(sl, ps[:, :qw],
                                                    linbb[:, :qw], Alu.add)
                        else:
                            nc.scalar.activation(sl, ps[:, :qw], Act.Copy)
                    nc.scalar.dma_start(
                        tab_dram[c0:c0 + w, :].rearrange(
                            "(g p) d -> p g d", p=P),
                        g[:, :w].rearrange("p (g d) -> p g d", d=P))

            # ---------------- Phase B: gather + scatter-add ----------------
            # chunk list: (lt, batch_off, cb, first_of_tile, last_of_tile)
            chunks = []
            gbase = 0
            for lt in range(NTC):
                nb = B[lt]
                done = 0
                while done < nb:
                    cb = min(GCHUNK, nb - done)
                    chunks.append((lt, gbase + done, cb, done == 0,
                                   done + cb == nb))
                    done += cb
                gbase += nb

            outT = fullp.tile([P, span], f32, tag="outT")
            h1 = fullp.tile([P, span], f32, tag="h")
            s_parts = smallp.tile([P, 2 * NTC], f32, tag="s_parts")
            gq = [0]
            gx_tiles = {}

            def emit_gx(j):
                lt, boff, cb, _, _ = chunks[j]
                ne = cb * P
                gx = gxp.tile([P, GCHUNK, D], bf16, tag="gx")
                nc.gpsimd.dma_gather(
                    gx[:, :cb, :], xw_tab[:],
                    srcw[:, boff * 8:boff * 8 + ne // 16],
                    ne, ne, D, single_packet=False, queue_num=gq[0] % NQ)
                gq[0] += 1
                gx_tiles[j] = gx

            # x-table, then early x-gathers overlap the e-table build
            build_table(xt_d, N2, wxt, xw_tab, False)
            LA = min(KLA, len(chunks))
            for j in range(LA):
                emit_gx(j)
            build_table(xbt_d, NB2, wbt, ew_tab, True)

            aggT = None
            for j, (lt, boff, cb, first_c, last_c) in enumerate(chunks):
                ne = cb * P
                if first_c:
                    aggT = psB.tile([P, P], f32, tag="aggT")
                ge = gep.tile([P, GCHUNK, D], bf16, tag="ge")
                nc.gpsimd.dma_gather(
                    ge[:, :cb, :], ew_tab[:],
                    briw[:, boff * 8:boff * 8 + ne // 16],
                    ne, ne, D, single_packet=False, queue_num=gq[0] % NQ)
                gq[0] += 1
                if j + LA < len(chunks):
                    emit_gx(j + LA)
                gx = gx_tiles.pop(j)

                nc.vector.tensor_tensor(gx[:, :cb, :], gx[:, :cb, :],
                                        ge[:, :cb, :], Alu.add)
                nc.scalar.activation(gx[:, :cb, :], gx[:, :cb, :], Act.Relu)

                oh = ohp.tile([P, GCHUNK * P], bf16, tag="oh")
                dl = dstloc[:, boff:boff + cb]
                dl_rep = dl.to_broadcast((P, cb, P))
                nc.vector.tensor_tensor(
                    oh[:, :cb * P].rearrange("p (b c) -> p b c", c=P),
                    iotab[:, :cb * P].rearrange("p (b c) -> p b c", c=P),
                    dl_rep, Alu.is_equal)
                for b in range(cb):
                    nc.tensor.matmul(
                        aggT[:], gx[:, b, :], oh[:, b * P:(b + 1) * P],
                        start=(first_c and b == 0), stop=(last_c and b == cb - 1))

                if last_c:
                    # residual + linear1 + stats + h1-out for this tile,
                    # all under later tiles' gathers
                    sl = slice(lt * P, (lt + 1) * P)
                    nc.vector.scalar_tensor_tensor(
                        outT[:, sl], xct[:, sl], float(resid_scale), aggT[:],
                        Alu.mult, Alu.add)
                    ps2 = psA.tile([P, P], f32, tag="psA")
                    nc.tensor.matmul(ps2[:], w1t[:], outT[:, sl])
                    nc.scalar.activation(h1[:, sl], ps2[:], Act.Copy)
                    nc.vector.reduce_sum(s_parts[:, 2 * lt:2 * lt + 1],
                                         h1[:, sl], axis=mybir.AxisListType.X)
                    sqt = smallp.tile([P, P], f32, tag="sqt")
                    nc.vector.tensor_tensor(sqt[:], h1[:, sl], h1[:, sl],
                                            Alu.mult)
                    nc.vector.reduce_sum(s_parts[:, 2 * lt + 1:2 * lt + 2],
                                         sqt[:], axis=mybir.AxisListType.X)
                    nc.sync.dma_start(h_out_d[:, sl], h1[:, sl])

            # ---------------- Phase C head: per-core BN stats --------
            s_stat = smallp.tile([P, 2], f32, tag="stat1")
            nc.vector.reduce_sum(
                s_stat[:, 0:1],
                s_parts[:].rearrange("p (t two) -> p t two", two=2)[:, :, 0:1],
                axis=mybir.AxisListType.XY)
            nc.vector.reduce_sum(
                s_stat[:, 1:2],
                s_parts[:].rearrange("p (t two) -> p t two", two=2)[:, :, 1:2],
                axis=mybir.AxisListType.XY)
            if not KFUSE:
                nc.sync.dma_start(stat_out_d[:], s_stat[:])
            else:
                # ---- fused tail: BN1 -> relu -> mask -> linear2 -> BN2 ----
                def allreduce_stats(idx, stat_tile):
                    nc.sync.dma_start(bn_in[idx][:], stat_tile[:])
                    nc.gpsimd.collective_compute(
                        "AllReduce", Alu.add,
                        replica_groups=[list(range(NCORES))],
                        ins=[bn_in[idx][:]], outs=[bn_out[idx][:]])
                    red = smallp.tile([P, 2], f32, tag=f"red{idx}")
                    nc.sync.dma_start(red[:], bn_out[idx][:])
                    return red

                red0 = allreduce_stats(0, s_stat)
                a0, sh0 = _bn_scale_shift(nc, mybir, smallp, red0, g1c, b1c, 0)
                # BN1-relu in place on h1, then mask
                nc.scalar.activation(h1[:], h1[:], Act.Relu,
                                     bias=sh0[:], scale=a0[:])
                nc.vector.tensor_tensor(h1[:], h1[:], maskb[:], Alu.mult)

                h2 = fullp.tile([P, span], f32, tag="h2")
                for lt in range(NTC):
                    sl = slice(lt * P, (lt + 1) * P)
                    ps = psA.tile([P, P], f32, tag="psA")
                    nc.tensor.matmul(ps[:], w2t[:], h1[:, sl])
                    nc.scalar.activation(h2[:, sl], ps[:], Act.Copy)
                s2 = smallp.tile([P, 2], f32, tag="stat2")
                nc.vector.reduce_sum(s2[:, 0:1], h2[:],
                                     axis=mybir.AxisListType.X)
                sq2 = fullp.tile([P, span], f32, tag="sq")
                nc.vector.tensor_tensor(sq2[:], h2[:], h2[:], Alu.mult)
                nc.vector.reduce_sum(s2[:, 1:2], sq2[:],
                                     axis=mybir.AxisListType.X)
                red1 = allreduce_stats(1, s2)
                a1, sh1 = _bn_scale_shift(nc, mybir, smallp, red1, g2c, b2c, 1)
                nc.scalar.activation(h2[:], h2[:], Act.Relu,
                                     bias=sh1[:], scale=a1[:])

                stg = fullp.tile([P, span], f32, tag="sq")
                for lt in range(NTC):
                    sl = slice(lt * P, (lt + 1) * P)
                    ps = psA.tile([P, P], f32, tag="psA")
                    nc.tensor.transpose(ps[:], h2[:, sl], ident[:])
                    nc.scalar.activation(stg[:, sl], ps[:], Act.Copy)
                nc.sync.dma_start(
                    out_d[:].rearrange("(g p) d -> p g d", p=P),
                    stg[:].rearrange("p (g d) -> p g d", d=P))

    nc.compile()
    return nc


_cache2 = {}


def _bn_scale_shift(nc, mybir, smallp, red, gcol, bcol, idx):
    """Device-side BN coefficients from reduced stats: a = g*rstd, sh = b - mu*a."""
    f32 = mybir.dt.float32
    Alu = mybir.AluOpType
    Act = mybir.ActivationFunctionType
    mu = smallp.tile([P, 1], f32, tag=f"mu{idx}")
    nc.vector.tensor_scalar(mu[:], red[:, 0:1], 1.0 / N, None, Alu.mult)
    mu2 = smallp.tile([P, 1], f32, tag=f"mu2{idx}")
    nc.vector.tensor_tensor(mu2[:], mu[:], mu[:], Alu.mult)
    e2 = smallp.tile([P, 1], f32, tag=f"e2{idx}")
    nc.vector.tensor_scalar(e2[:], red[:, 1:2], 1.0 / N, None, Alu.mult)
    var = smallp.tile([P, 1], f32, tag=f"var{idx}")
    nc.vector.tensor_tensor(var[:], e2[:], mu2[:], Alu.subtract)
    vep = smallp.tile([P, 1], f32, tag=f"vep{idx}")
    nc.vector.tensor_scalar(vep[:], var[:], BN_EPS, None, Alu.add)
    std = smallp.tile([P, 1], f32, tag=f"std{idx}")
    nc.scalar.activation(std[:], vep[:], Act.Sqrt)
    rstd = smallp.tile([P, 1], f32, tag=f"rstd{idx}")
    nc.vector.reciprocal(rstd[:], std[:])
    a = smallp.tile([P, 1], f32, tag=f"a{idx}")
    nc.vector.tensor_tensor(a[:], gcol[:], rstd[:], Alu.mult)
    tmp = smallp.tile([P, 1], f32, tag=f"tmp{idx}")
    nc.vector.tensor_tensor(tmp[:], mu[:], a[:], Alu.mult)
    sh = smallp.tile([P, 1], f32, tag=f"sh{idx}")
    nc.vector.tensor_tensor(sh[:], bcol[:], tmp[:], Alu.subtract)
    return a, sh


def _build_phase2():
    """NEFF2: h1n = mask*relu(BN1(h1)); h2 = h1n @ w2.T; per-core stats of h2."""
    import concourse.bacc as bacc
    import concourse.mybir as mybir
    import concourse.tile as tile

    f32 = mybir.dt.float32
    Alu = mybir.AluOpType
    Act = mybir.ActivationFunctionType
    span = NTC * P

    nc = bacc.Bacc("TRN2", target_bir_lowering=False, debug=False,
                   num_devices=NCORES)
    nc.sbuf_top = min(nc.sbuf_top, 192 * 1024)

    def din(name, shape):
        return nc.dram_tensor(name, shape, f32, kind="ExternalInput")

    h_d = din("h_in", [P, span])
    red_d = din("red", [P, 2])
    w2t_d = din("w2t", [D, D])
    g1c_d = din("g1c", [P, 1]); b1c_d = din("b1c", [P, 1])
    maskb_d = din("maskb", [P, span])
    h_out_d = nc.dram_tensor("h_out", [P, span], f32, kind="ExternalOutput")
    stat_out_d = nc.dram_tensor("stat_out", [P, 2], f32, kind="ExternalOutput")

    with tile.TileContext(nc) as tc:
        with (
            tc.tile_pool(name="consts", bufs=1) as cp,
            tc.tile_pool(name="psA", bufs=2, space="PSUM") as psA,
            tc.tile_pool(name="full", bufs=1) as fullp,
            tc.tile_pool(name="small", bufs=1) as smallp,
        ):
            def load_const(name, dram, shape):
                t = cp.tile(shape, f32, tag=f"c_{name}")
                nc.sync.dma_start(t[:], dram[:])
                return t

            h1 = load_const("h", h_d, [P, span])
            red = load_const("red", red_d, [P, 2])
            w2t = load_const("w2t", w2t_d, [D, D])
            g1c = load_const("g1c", g1c_d, [P, 1])
            b1c = load_const("b1c", b1c_d, [P, 1])
            maskb = load_const("maskb", maskb_d, [P, span])

            a, sh = _bn_scale_shift(nc, mybir, smallp, red, g1c, b1c, 0)
            hn = fullp.tile([P, span], f32, tag="hn")
            nc.scalar.activation(hn[:], h1[:], Act.Relu, bias=sh[:], scale=a[:])
            nc.vector.tensor_tensor(hn[:], hn[:], maskb[:], Alu.mult)

            h2 = fullp.tile([P, span], f32, tag="h2")
            for lt in range(NTC):
                sl = slice(lt * P, (lt + 1) * P)
                ps = psA.tile([P, P], f32, tag="psA")
                nc.tensor.matmul(ps[:], w2t[:], hn[:, sl])
                nc.scalar.activation(h2[:, sl], ps[:], Act.Copy)
            s_stat = smallp.tile([P, 2], f32, tag="stat2")
            nc.vector.reduce_sum(s_stat[:, 0:1], h2[:],
                                 axis=mybir.AxisListType.X)
            sq = fullp.tile([P, span], f32, tag="sq")
            nc.vector.tensor_tensor(sq[:], h2[:], h2[:], Alu.mult)
            nc.vector.reduce_sum(s_stat[:, 1:2], sq[:],
                                 axis=mybir.AxisListType.X)
            nc.sync.dma_start(stat_out_d[:], s_stat[:])
            nc.sync.dma_start(h_out_d[:], h2[:])

    nc.compile()
    return nc


def _build_phase3():
    """NEFF3: out = transpose(relu(BN2(h2)))."""
    import concourse.bacc as bacc
    import concourse.mybir as mybir
    import concourse.tile as tile

    f32 = mybir.dt.float32
    Act = mybir.ActivationFunctionType
    span = NTC * P

    nc = bacc.Bacc("TRN2", target_bir_lowering=False, debug=False,
                   num_devices=NCORES)
    nc.sbuf_top = min(nc.sbuf_top, 192 * 1024)

    def din(name, shape):
        return nc.dram_tensor(name, shape, f32, kind="ExternalInput")

    h_d = din("h_in", [P, span])
    red_d = din("red", [P, 2])
    g2c_d = din("g2c", [P, 1]); b2c_d = din("b2c", [P, 1])
    ident_d = din("ident", [P, P])
    out_d = nc.dram_tensor("out", [span, D], f32, kind="ExternalOutput")

    with tile.TileContext(nc) as tc:
        with (
            tc.tile_pool(name="consts", bufs=1) as cp,
            tc.tile_pool(name="psA", bufs=2, space="PSUM") as psA,
            tc.tile_pool(name="full", bufs=1) as fullp,
            tc.tile_pool(name="small", bufs=1) as smallp,
        ):
            def load_const(name, dram, shape):
                t = cp.tile(shape, f32, tag=f"c_{name}")
                nc.sync.dma_start(t[:], dram[:])
                return t

            h2 = load_const("h", h_d, [P, span])
            red = load_const("red", red_d, [P, 2])
            g2c = load_const("g2c", g2c_d, [P, 1])
            b2c = load_const("b2c", b2c_d, [P, 1])
            ident = load_const("ident", ident_d, [P, P])

            a, sh = _bn_scale_shift(nc, mybir, smallp, red, g2c, b2c, 1)
            hn = fullp.tile([P, span], f32, tag="hn")
            nc.scalar.activation(hn[:], h2[:], Act.Relu, bias=sh[:], scale=a[:])

            stg = fullp.tile([P, span], f32, tag="stg")
            for lt in range(NTC):
                sl = slice(lt * P, (lt + 1) * P)
                ps = psA.tile([P, P], f32, tag="psA")
                nc.tensor.transpose(ps[:], hn[:, sl], ident[:])
                nc.scalar.activation(stg[:, sl], ps[:], Act.Copy)
            nc.sync.dma_start(
                out_d[:].rearrange("(g p) d -> p g d", p=P),
                stg[:].rearrange("p (g d) -> p g d", d=P))

    nc.compile()
    return nc


def _build_phase23():
    """Single second NEFF: every core gets the full (rotated) bf16 h1 with
    its own span at columns [0:span] and exact zeros appended for the 240
    padded nodes. BN1-apply + linear2 + global BN2 stats are computed
    replicated (mask-free, exact); BN2-apply + transpose + output only for
    the own span. Removes one NEFF launch and one host sync."""
    import concourse.bacc as bacc
    import concourse.mybir as mybir
    import concourse.tile as tile

    f32 = mybir.dt.float32
    bf16 = mybir.dt.bfloat16
    Alu = mybir.AluOpType
    Act = mybir.ActivationFunctionType
    span = NTC * P
    FULL = N2  # 10240 = 10000 real + 240 zero columns

    nc = bacc.Bacc("TRN2", target_bir_lowering=False, debug=False,
                   num_devices=NCORES)
    nc.sbuf_top = min(nc.sbuf_top, 192 * 1024)

    h_d = nc.dram_tensor("h_all", [P, FULL], bf16, kind="ExternalInput")
    red_d = nc.dram_tensor("red", [P, 2], f32, kind="ExternalInput")
    w2tb_d = nc.dram_tensor("w2tb", [D, D], bf16, kind="ExternalInput")
    g1c_d = nc.dram_tensor("g1c", [P, 1], f32, kind="ExternalInput")
    b1c_d = nc.dram_tensor("b1c", [P, 1], f32, kind="ExternalInput")
    g2c_d = nc.dram_tensor("g2c", [P, 1], f32, kind="ExternalInput")
    b2c_d = nc.dram_tensor("b2c", [P, 1], f32, kind="ExternalInput")
    ident_d = nc.dram_tensor("identb", [P, P], bf16, kind="ExternalInput")
    out_d = nc.dram_tensor("out", [span, D], f32, kind="ExternalOutput")

    with tile.TileContext(nc) as tc:
        with (
            tc.tile_pool(name="consts", bufs=1) as cp,
            tc.tile_pool(name="ps", bufs=2, space="PSUM") as psp,
            tc.tile_pool(name="full", bufs=1) as fullp,
            tc.tile_pool(name="small", bufs=1) as smallp,
        ):
            def load_const(name, dram, shape, dt=f32):
                t = cp.tile(shape, dt, tag=f"c_{name}")
                nc.sync.dma_start(t[:], dram[:])
                return t

            h1 = load_const("h", h_d, [P, FULL], bf16)
            red = load_const("red", red_d, [P, 2])
            w2tb = load_const("w2tb", w2tb_d, [D, D], bf16)
            g1c = load_const("g1c", g1c_d, [P, 1])
            b1c = load_const("b1c", b1c_d, [P, 1])
            g2c = load_const("g2c", g2c_d, [P, 1])
            b2c = load_const("b2c", b2c_d, [P, 1])
            ident = load_const("ident", ident_d, [P, P], bf16)

            a0, sh0 = _bn_scale_shift(nc, mybir, smallp, red, g1c, b1c, 0)
            # BN1-apply + relu in place (bf16); zero columns stay relu(sh0)
            # only in the 240 appended pad columns... they must be zeroed
            # for exact stats: recompute them as zeros via memset after.
            nc.scalar.activation(h1[:], h1[:], Act.Relu,
                                 bias=sh0[:], scale=a0[:])
            nc.vector.memset(h1[:, N:FULL], 0.0)

            # linear2 over the full width, bf16, w2 stationary
            h2 = fullp.tile([P, FULL], bf16, tag="h2")
            for q0 in range(0, FULL, 512):
                ps = psp.tile([P, 512], f32, tag="ps")
                for j in range(4):
                    nc.tensor.matmul(ps[:, j * P:(j + 1) * P], w2tb[:],
                                     h1[:, q0 + j * P:q0 + (j + 1) * P])
                nc.scalar.activation(h2[:, q0:q0 + 512], ps[:], Act.Copy)

            # global BN2 stats (exact: pad columns of h2 = W2 @ 0 = 0)
            s2 = smallp.tile([P, 2], f32, tag="s2")
            nc.vector.reduce_sum(s2[:, 0:1], h2[:], axis=mybir.AxisListType.X)
            sq = fullp.tile([P, FULL], f32, tag="sq")
            nc.vector.tensor_tensor(sq[:], h2[:], h2[:], Alu.mult)
            nc.vector.reduce_sum(s2[:, 1:2], sq[:], axis=mybir.AxisListType.X)
            a1, sh1 = _bn_scale_shift(nc, mybir, smallp, s2, g2c, b2c, 1)

            # BN2-apply + relu + transpose for the own span only
            hn2 = fullp.tile([P, span], bf16, tag="hn2")
            nc.scalar.activation(hn2[:], h2[:, :span], Act.Relu,
                                 bias=sh1[:], scale=a1[:])
            stg = fullp.tile([P, span], f32, tag="stg")
            for lt in range(NTC):
                sl = slice(lt * P, (lt + 1) * P)
                ps = psp.tile([P, P], bf16, tag="pst")
                nc.tensor.transpose(ps[:], hn2[:, sl], ident[:])
                nc.scalar.activation(stg[:, sl], ps[:], Act.Copy)
            nc.sync.dma_start(
                out_d[:].rearrange("(g p) d -> p g d", p=P),
                stg[:].rearrange("p (g d) -> p g d", d=P))

    nc.compile()
    return nc


def kernel(**inputs):
    global last_results
    from concourse.bass_utils import run_bass_kernel_spmd

    in_maps, meta = _host_prep(inputs)
    if meta not in _cache:
        _cache[meta] = _build(meta)
    if not KFUSE and KP23 and "p23" not in _cache2:
        _cache2["p23"] = _build_phase23()
    if not KFUSE and not KP23 and "p2" not in _cache2:
        _cache2["p2"] = _build_phase2()
        _cache2["p3"] = _build_phase3()
    nc1 = _cache[meta]
    nc2 = _cache2.get("p2"); nc3 = _cache2.get("p3")
    cores = list(range(NCORES))
    trace = bool(os.environ.get("KERNEL_TRACE"))

    n1_keys = ["xt", "xbt", "wxt", "wbt", "w1t", "linbb", "iotab",
               "xct", "srcw", "briw", "dstloc"]
    if KFUSE:
        n1_keys += ["w2t", "g1c", "b1c", "g2c", "b2c", "maskb", "ident"]
        in1 = [{k: in_maps[c][k] for k in n1_keys} for c in range(NCORES)]
        res1 = run_bass_kernel_spmd(nc1, in1, cores, trace=trace)
        last_results = (res1,)
        out = np.concatenate([res1.results[c]["out"] for c in range(NCORES)],
                             axis=0)
        return np.ascontiguousarray(out[:N])
    in1 = [{k: in_maps[c][k] for k in n1_keys} for c in range(NCORES)]
    res1 = run_bass_kernel_spmd(nc1, in1, cores, trace=trace)
    red1 = np.sum([res1.results[c]["stat_out"] for c in range(NCORES)], axis=0)
    if KP23:
        span = NTC * P
        hb = np.concatenate([res1.results[c]["h_out"] for c in range(NCORES)],
                            axis=1).astype(BF16)  # [P, 10240]
        zpad = np.zeros((P, N2 - N), BF16)
        in2 = []
        for c in range(NCORES):
            roll = np.concatenate(
                [hb[:, span * c:N], hb[:, :span * c], zpad], axis=1)
            in2.append({"h_all": np.ascontiguousarray(roll), "red": red1,
                        "w2tb": in_maps[c]["w2tb"],
                        "identb": in_maps[c]["identb"],
                        "g1c": in_maps[c]["g1c"], "b1c": in_maps[c]["b1c"],
                        "g2c": in_maps[c]["g2c"], "b2c": in_maps[c]["b2c"]})
        res2 = run_bass_kernel_spmd(_cache2["p23"], in2, cores, trace=trace)
        last_results = (res1, res2)
        out = np.concatenate([res2.results[c]["out"] for c in range(NCORES)],
                             axis=0)
        return np.ascontiguousarray(out[:N])
    in2 = [{"h_in": res1.results[c]["h_out"], "red": red1,
            "w2t": in_maps[c]["w2t"], "g1c": in_maps[c]["g1c"],
            "b1c": in_maps[c]["b1c"], "maskb": in_maps[c]["maskb"]}
           for c in range(NCORES)]
    res2 = run_bass_kernel_spmd(nc2, in2, cores, trace=trace)
    red2 = np.sum([res2.results[c]["stat_out"] for c in range(NCORES)], axis=0)
    in3 = [{"h_in": res2.results[c]["h_out"], "red": red2,
            "g2c": in_maps[c]["g2c"], "b2c": in_maps[c]["b2c"],
            "ident": in_maps[c]["ident"]} for c in range(NCORES)]
    res3 = run_bass_kernel_spmd(nc3, in3, cores, trace=trace)

    last_results = (res1, res2, res3)
    out = np.concatenate([res3.results[c]["out"] for c in range(NCORES)], axis=0)
    return np.ascontiguousarray(out[:N])


# revision 31
# speedup vs baseline: 1.0661x; 1.0022x over previous
"""Trainium2 Bass kernel for nn_AdjacencyConv (GNN message passing).

Reference computation:
    msg  = relu(concat[x[src], x_bridge[bri]] @ lin_w.T + lin_b)   # [E, D]
    agg  = segment_sum(msg, dst, N)                                # [N, D]
    out  = agg + (1+eps)*x
    h    = relu(BN(out @ w1.T + b1)); h = relu(BN(h @ w2.T + b2))  # train-mode BN

Device algorithm (8-core SPMD, edges sharded by dst node-tile):
  Phase A (per core, replicated): build bf16 node tables in DRAM
        xw_tab = x @ Wx.T            (Wx = lin_w[:, :D])
        ew_tab = x_bridge @ Wb.T + b (Wb = lin_w[:, D:])
    so the per-edge linear factorizes: msg = relu(xw_tab[src] + ew_tab[bri]).
    The x-table is built first; the first KLA chunks' x-gathers are emitted
    before the e-table build so SWDGE descriptor generation (the kernel's
    bottleneck, ~3ns/idx serial on the GpSimd engine) overlaps phase A.
  Phase B: per dst node-tile of 128 nodes, per chunk of GCHUNK 128-edge
    batches: dma_gather x-rows and e-rows (bf16, 256B descriptors,
    4 SWDGE queues, x-gathers KLA chunks ahead) edge-major [128e, cb, D],
    add + relu in bf16, scatter-add via bf16 one-hot matmuls accumulating
    feature-major agg in PSUM (f32). Residual, linear1, per-tile BN stats
    and the h1 output DMA all run per-tile under later tiles' gathers.
  Phase C: BN stats are reduced across cores host-side between NEFFs
    (biases b1/b2 cancel in BN and are dropped; on-device AllReduce was
    measured ~170us per collective here — host reduction is faster).
    NEFF2 = BN1+relu+linear2, NEFF3 = BN2+relu+transpose to node-major.

Host side does only layout preprocessing: index sorting/padding/packing,
dtype casts, weight transposes, and output assembly.
"""

import os
import numpy as np
import ml_dtypes

BF16 = ml_dtypes.bfloat16

N, NB, E, D = 10000, 20000, 640000, 128
P = 128
NCORES = 8
NTC = 10                    # node tiles per core
N2 = NCORES * NTC * P       # 10240 padded nodes
NB2 = 20096                 # x_bridge padded to multiple of 128
R2 = N2 + NB2               # combined table rows
BN_EPS = 1e-5
GCHUNK = int(os.environ.get("KGCHUNK", "16"))  # batches per dma_gather
NQ = int(os.environ.get("KNQ", "4"))           # SWDGE queues
KSCRATCH = int(os.environ.get("KSCRATCH", "16384"))
KGB = int(os.environ.get("KGB", "3"))          # gather pool bufs
KFUSE = int(os.environ.get("KFUSE", "0"))      # 1 = single fused NEFF
KLA = int(os.environ.get("KLA", "10"))          # x-gather lookahead chunks
KP23 = int(os.environ.get("KP23", "0"))        # merged phase-2/3 NEFF

_cache = {}

# exposed for test.py
last_results = None


def _pack_idx(idx16):
    """Wrap an int16 index vector for dma_gather: [16, n/16] replicated x8."""
    w = idx16.reshape(-1, 16).T
    return np.tile(w, (8, 1)).copy()


def _host_prep(inputs):
    x = np.asarray(inputs["x"], np.float32)
    xb = np.asarray(inputs["x_bridge"], np.float32)
    ei = np.asarray(inputs["edge_index"])
    bri = np.asarray(inputs["bridge_index"])
    lin_w = np.asarray(inputs["lin_w"], np.float32)
    lin_b = np.asarray(inputs["lin_b"], np.float32)
    eps = float(np.asarray(inputs["eps"]).reshape(-1)[0])
    w1 = np.asarray(inputs["w1"], np.float32)
    g1 = np.asarray(inputs["g1"], np.float32)
    beta1 = np.asarray(inputs["beta1"], np.float32)
    w2 = np.asarray(inputs["w2"], np.float32)
    g2 = np.asarray(inputs["g2"], np.float32)
    beta2 = np.asarray(inputs["beta2"], np.float32)

    src = ei[0].astype(np.int64)
    dst = ei[1].astype(np.int64)
    bri = bri.astype(np.int64)

    # sort edges by dst, bucket into 128-node tiles
    order = np.argsort(dst, kind="stable")
    dsts = dst[order]
    srcs = src[order]
    bris = bri[order]
    gt_bounds = np.searchsorted(dsts, np.arange(NCORES * NTC + 1) * P)

    # uniform program structure: per local tile lt, same batch count across cores
    cnt = np.diff(gt_bounds)  # edges per global tile (len 80)
    cnt = cnt.reshape(NCORES, NTC)
    ceils = -(-cnt // P)
    B = np.maximum(1, ceils.max(axis=0))  # batches per local tile (len NTC)
    SB = int(B.sum())          # total batches per core
    S = SB * P                 # padded edges per core

    src_pad = np.zeros((NCORES, S), np.int64)
    bri_pad = np.zeros((NCORES, S), np.int64)
    dloc_pad = np.full((NCORES, S), 999.0, np.float32)
    for c in range(NCORES):
        off = 0
        for lt in range(NTC):
            gt = c * NTC + lt
            a, b = gt_bounds[gt], gt_bounds[gt + 1]
            n = b - a
            src_pad[c, off:off + n] = srcs[a:b]
            bri_pad[c, off:off + n] = bris[a:b]
            dloc_pad[c, off:off + n] = (dsts[a:b] - gt * P).astype(np.float32)
            off += B[lt] * P

    # dstloc transposed: [128, SB], column j = batch j's 128 local-dst values
    dloc_T = np.ascontiguousarray(
        dloc_pad.reshape(NCORES, SB, P).transpose(0, 2, 1))

    # feature-major bf16 padded inputs for the table build
    xt = np.zeros((D, N2), BF16)
    xt[:, :N] = x.T.astype(BF16)
    xbt = np.zeros((D, NB2), BF16)
    xbt[:, :NB] = xb.T.astype(BF16)

    wxt = np.ascontiguousarray(lin_w[:, :D].T).astype(BF16)   # [in_f, out]
    wbt = np.ascontiguousarray(lin_w[:, D:].T).astype(BF16)
    w1t = np.ascontiguousarray(w1.T)
    w2t = np.ascontiguousarray(w2.T)
    w2tb = w2t.astype(BF16)
    identb = np.eye(P, dtype=BF16)
    linbb = np.tile(lin_b[None, :], (P, 4)).astype(np.float32)   # [128, 512]
    iotab = np.tile(np.arange(P, dtype=np.float32)[None, :],
                    (P, GCHUNK)).astype(BF16)  # [128, GCHUNK*128]
    ident = np.eye(P, dtype=np.float32)

    g1c = np.ascontiguousarray(g1[:, None])
    b1c = np.ascontiguousarray(beta1[:, None])
    g2c = np.ascontiguousarray(g2[:, None])
    b2c = np.ascontiguousarray(beta2[:, None])

    # per-core residual slice (feature-major) and validity mask
    span = NTC * P
    xct = np.zeros((NCORES, D, span), np.float32)
    maskb = np.zeros((NCORES, P, span), np.float32)
    for c in range(NCORES):
        c0 = c * span
        v = min(max(N - c0, 0), span)
        if v > 0:
            xct[c, :, :v] = x.T[:, c0:c0 + v]
            maskb[c, :, :v] = 1.0

    in_maps = []
    for c in range(NCORES):
        in_maps.append({
            "xt": xt, "xbt": xbt,
            "wxt": wxt, "wbt": wbt, "w1t": w1t, "w2t": w2t,
            "w2tb": w2tb, "identb": identb,
            "linbb": linbb, "iotab": iotab, "ident": ident,
            "g1c": g1c, "b1c": b1c, "g2c": g2c, "b2c": b2c,
            "xct": np.ascontiguousarray(xct[c]),
            "maskb": np.ascontiguousarray(maskb[c]).astype(BF16),
            "srcw": _pack_idx(src_pad[c].astype(np.int16)),
            "briw": _pack_idx(bri_pad[c].astype(np.int16)),
            "dstloc": np.ascontiguousarray(dloc_T[c].astype(BF16)),
        })
    meta = (tuple(int(b) for b in B), 1.0 + eps)
    return in_maps, meta


def _build(meta):
    import concourse.bacc as bacc
    import concourse.mybir as mybir
    import concourse.tile as tile

    B, resid_scale = meta
    SB = sum(B)
    S = SB * P
    f32 = mybir.dt.float32
    bf16 = mybir.dt.bfloat16
    i16 = mybir.dt.int16
    Alu = mybir.AluOpType
    Act = mybir.ActivationFunctionType
    span = NTC * P

    nc = bacc.Bacc("TRN2", target_bir_lowering=False, debug=False,
                   num_devices=NCORES, num_swdge_queues=NQ,
                   dynamic_dma_scratch_size=KSCRATCH)
    # Leave the top of SBUF for the runtime's SWDGE descriptor rings /
    # DynamicDMAScratch carveout — allocating into it wedges the device.
    nc.sbuf_top = min(nc.sbuf_top, 192 * 1024)

    def din(name, shape, dt=f32):
        return nc.dram_tensor(name, shape, dt, kind="ExternalInput")

    xt_d = din("xt", [D, N2], bf16); xbt_d = din("xbt", [D, NB2], bf16)
    wxt_d = din("wxt", [D, D], bf16); wbt_d = din("wbt", [D, D], bf16)
    w1t_d = din("w1t", [D, D])
    linbb_d = din("linbb", [P, 512]); iotab_d = din("iotab", [P, GCHUNK * P], bf16)
    xct_d = din("xct", [D, span])
    srcw_d = din("srcw", [128, S // 16], i16)
    briw_d = din("briw", [128, S // 16], i16)
    dstloc_d = din("dstloc", [P, SB], bf16)
    if KFUSE:
        w2t_d = din("w2t", [D, D])
        g1c_d = din("g1c", [P, 1]); b1c_d = din("b1c", [P, 1])
        g2c_d = din("g2c", [P, 1]); b2c_d = din("b2c", [P, 1])
        maskb_d = din("maskb", [P, span])
        ident_d = din("ident", [P, P])
        out_d = nc.dram_tensor("out", [span, D], f32, kind="ExternalOutput")
        bn_in = [nc.dram_tensor(f"bn_in{i}", [P, 2], f32) for i in range(2)]
        bn_out = [nc.dram_tensor(f"bn_out{i}", [P, 2], f32,
                                 addr_space="Shared") for i in range(2)]
    else:
        h_out_d = nc.dram_tensor("h_out", [P, span], f32, kind="ExternalOutput")
        stat_out_d = nc.dram_tensor("stat_out", [P, 2], f32,
                                    kind="ExternalOutput")

    xw_tab = nc.dram_tensor("xw_tab", [N2, D], bf16)
    ew_tab = nc.dram_tensor("ew_tab", [NB2, D], bf16)
    with tile.TileContext(nc) as tc:
        with (
            tc.tile_pool(name="consts", bufs=1) as cp,
            tc.tile_pool(name="pa_src", bufs=3) as pa_src,
            tc.tile_pool(name="pa_stg", bufs=3) as pa_stg,
            tc.tile_pool(name="psA", bufs=2, space="PSUM") as psA,
            tc.tile_pool(name="psB", bufs=2, space="PSUM") as psB,
            tc.tile_pool(name="gx", bufs=KLA + 2) as gxp,
            tc.tile_pool(name="ge", bufs=3) as gep,
            tc.tile_pool(name="oh", bufs=3) as ohp,
            tc.tile_pool(name="full", bufs=1) as fullp,
            tc.tile_pool(name="small", bufs=1) as smallp,
        ):
            def load_const(name, dram, shape, dt=f32):
                t = cp.tile(shape, dt, tag=f"c_{name}")
                nc.sync.dma_start(t[:], dram[:])
                return t

            wxt = load_const("wxt", wxt_d, [D, D], bf16)
            wbt = load_const("wbt", wbt_d, [D, D], bf16)
            w1t = load_const("w1t", w1t_d, [D, D])
            linbb = load_const("linbb", linbb_d, [P, 512])
            iotab = load_const("iotab", iotab_d, [P, GCHUNK * P], bf16)
            xct = load_const("xct", xct_d, [D, span])
            srcw = load_const("srcw", srcw_d, [128, S // 16], i16)
            briw = load_const("briw", briw_d, [128, S // 16], i16)
            dstloc = load_const("dstloc", dstloc_d, [P, SB], bf16)
            if KFUSE:
                w2t = load_const("w2t", w2t_d, [D, D])
                g1c = load_const("g1c", g1c_d, [P, 1])
                b1c = load_const("b1c", b1c_d, [P, 1])
                g2c = load_const("g2c", g2c_d, [P, 1])
                b2c = load_const("b2c", b2c_d, [P, 1])
                maskb = load_const("maskb", maskb_d, [P, span])
                ident = load_const("ident", ident_d, [P, P])

            # ---------------- Phase A: combined bf16 node table ----------------
            CW = 4096  # source columns per chunk

            def build_table(src_dram, ncols, w_sbuf, tab_dram, add_bias,
                            cw=CW):
                for c0 in range(0, ncols, cw):
                    w = min(cw, ncols - c0)
                    s = pa_src.tile([D, CW], bf16, tag="pa_src")
                    nc.sync.dma_start(s[:, :w], src_dram[:, c0:c0 + w])
                    g = pa_stg.tile([P, CW], bf16, tag="pa_stg")
                    for q0 in range(0, w, 512):
                        qw = min(512, w - q0)
                        ps = psA.tile([P, 512], f32, tag="psAb")
                        for j in range(qw // P):
                            nc.tensor.matmul(
                                ps[:, j * P:(j + 1) * P],
                                s[:, q0 + j * P:q0 + (j + 1) * P], w_sbuf[:])
                        sl = g[:, q0:q0 + qw]
                        if add_bias:
                            nc.vector.tensor_tensor(sl, ps[:, :qw],
                                                    linbb[:, :qw], Alu.add)
                        else:
                            nc.scalar.activation(sl, ps[:, :qw], Act.Copy)
                    nc.scalar.dma_start(
                        tab_dram[c0:c0 + w, :].rearrange(
                            "(g p) d -> p g d", p=P),
                        g[:, :w].rearrange("p (g d) -> p g d", d=P))

            # ---------------- Phase B: gather + scatter-add ----------------
            # chunk list: (lt, batch_off, cb, first_of_tile, last_of_tile)
            chunks = []
            gbase = 0
            for lt in range(NTC):
                nb = B[lt]
                done = 0
                while done < nb:
                    cb = min(GCHUNK, nb - done)
                    chunks.append((lt, gbase + done, cb, done == 0,
                                   done + cb == nb))
                    done += cb
                gbase += nb

            outT = fullp.tile([P, span], f32, tag="outT")
            h1 = fullp.tile([P, span], f32, tag="h")
            s_parts = smallp.tile([P, 2 * NTC], f32, tag="s_parts")
            gq = [0]
            gx_tiles = {}

            def emit_gx(j):
                lt, boff, cb, _, _ = chunks[j]
                ne = cb * P
                gx = gxp.tile([P, GCHUNK, D], bf16, tag="gx")
                nc.gpsimd.dma_gather(
                    gx[:, :cb, :], xw_tab[:],
                    srcw[:, boff * 8:boff * 8 + ne // 16],
                    ne, ne, D, single_packet=False, queue_num=gq[0] % NQ)
                gq[0] += 1
                gx_tiles[j] = gx

            # x-table, then early x-gathers overlap the e-table build
            build_table(xt_d, N2, wxt, xw_tab, False)
            LA = min(KLA, len(chunks))
            for j in range(LA):
                emit_gx(j)
            build_table(xbt_d, NB2, wbt, ew_tab, True)

            aggT = None
            for j, (lt, boff, cb, first_c, last_c) in enumerate(chunks):
                ne = cb * P
                if first_c:
                    aggT = psB.tile([P, P], f32, tag="aggT")
                ge = gep.tile([P, GCHUNK, D], bf16, tag="ge")
                nc.gpsimd.dma_gather(
                    ge[:, :cb, :], ew_tab[:],
                    briw[:, boff * 8:boff * 8 + ne // 16],
                    ne, ne, D, single_packet=False, queue_num=gq[0] % NQ)
                gq[0] += 1
                if j + LA < len(chunks):
                    emit_gx(j + LA)
                gx = gx_tiles.pop(j)

                nc.vector.tensor_tensor(gx[:, :cb, :], gx[:, :cb, :],
                                        ge[:, :cb, :], Alu.add)
                nc.scalar.activation(gx[:, :cb, :], gx[:, :cb, :], Act.Relu)

                oh = ohp.tile([P, GCHUNK * P], bf16, tag="oh")
                dl = dstloc[:, boff:boff + cb]
                dl_rep = dl.to_broadcast((P, cb, P))
                nc.vector.tensor_tensor(
                    oh[:, :cb * P].rearrange("p (b c) -> p b c", c=P),
                    iotab[:, :cb * P].rearrange("p (b c) -> p b c", c=P),
                    dl_rep, Alu.is_equal)
                for b in range(cb):
                    nc.tensor.matmul(
                        aggT[:], gx[:, b, :], oh[:, b * P:(b + 1) * P],
                        start=(first_c and b == 0), stop=(last_c and b == cb - 1))

                if last_c:
                    # residual + linear1 + stats + h1-out for this tile,
                    # all under later tiles' gathers
                    sl = slice(lt * P, (lt + 1) * P)
                    nc.vector.scalar_tensor_tensor(
                        outT[:, sl], xct[:, sl], float(resid_scale), aggT[:],
                        Alu.mult, Alu.add)
                    ps2 = psA.tile([P, P], f32, tag="psA")
                    nc.tensor.matmul(ps2[:], w1t[:], outT[:, sl])
                    nc.scalar.activation(h1[:, sl], ps2[:], Act.Copy)
                    nc.vector.reduce_sum(s_parts[:, 2 * lt:2 * lt + 1],
                                         h1[:, sl], axis=mybir.AxisListType.X)
                    sqt = smallp.tile([P, P], f32, tag="sqt")
                    nc.vector.tensor_tensor(sqt[:], h1[:, sl], h1[:, sl],
                                            Alu.mult)
                    nc.vector.reduce_sum(s_parts[:, 2 * lt + 1:2 * lt + 2],
                                         sqt[:], axis=mybir.AxisListType.X)
                    nc.sync.dma_start(h_out_d[:, sl], h1[:, sl])

            # ---------------- Phase C head: per-core BN stats --------
            s_stat = smallp.tile([P, 2], f32, tag="stat1")
            nc.vector.reduce_sum(
                s_stat[:, 0:1],
                s_parts[:].rearrange("p (t two) -> p t two", two=2)[:, :, 0:1],
                axis=mybir.AxisListType.XY)
            nc.vector.reduce_sum(
                s_stat[:, 1:2],
                s_parts[:].rearrange("p (t two) -> p t two", two=2)[:, :, 1:2],
                axis=mybir.AxisListType.XY)
            if not KFUSE:
                nc.sync.dma_start(stat_out_d[:], s_stat[:])
            else:
                # ---- fused tail: BN1 -> relu -> mask -> linear2 -> BN2 ----
                def allreduce_stats(idx, stat_tile):
                    nc.sync.dma_start(bn_in[idx][:], stat_tile[:])
                    nc.gpsimd.collective_compute(
                        "AllReduce", Alu.add,
                        replica_groups=[list(range(NCORES))],
                        ins=[bn_in[idx][:]], outs=[bn_out[idx][:]])
                    red = smallp.tile([P, 2], f32, tag=f"red{idx}")
                    nc.sync.dma_start(red[:], bn_out[idx][:])
                    return red

                red0 = allreduce_stats(0, s_stat)
                a0, sh0 = _bn_scale_shift(nc, mybir, smallp, red0, g1c, b1c, 0)
                # BN1-relu in place on h1, then mask
                nc.scalar.activation(h1[:], h1[:], Act.Relu,
                                     bias=sh0[:], scale=a0[:])
                nc.vector.tensor_tensor(h1[:], h1[:], maskb[:], Alu.mult)

                h2 = fullp.tile([P, span], f32, tag="h2")
                for lt in range(NTC):
                    sl = slice(lt * P, (lt + 1) * P)
                    ps = psA.tile([P, P], f32, tag="psA")
                    nc.tensor.matmul(ps[:], w2t[:], h1[:, sl])
                    nc.scalar.activation(h2[:, sl], ps[:], Act.Copy)
                s2 = smallp.tile([P, 2], f32, tag="stat2")
                nc.vector.reduce_sum(s2[:, 0:1], h2[:],
                                     axis=mybir.AxisListType.X)
                sq2 = fullp.tile([P, span], f32, tag="sq")
                nc.vector.tensor_tensor(sq2[:], h2[:], h2[:], Alu.mult)
                nc.vector.reduce_sum(s2[:, 1:2], sq2[:],
                                     axis=mybir.AxisListType.X)
                red1 = allreduce_stats(1, s2)
                a1, sh1 = _bn_scale_shift(nc, mybir, smallp, red1, g2c, b2c, 1)
                nc.scalar.activation(h2[:], h2[:], Act.Relu,
                                     bias=sh1[:], scale=a1[:])

                stg = fullp.tile([P, span], f32, tag="sq")
                for lt in range(NTC):
                    sl = slice(lt * P, (lt + 1) * P)
                    ps = psA.tile([P, P], f32, tag="psA")
                    nc.tensor.transpose(ps[:], h2[:, sl], ident[:])
                    nc.scalar.activation(stg[:, sl], ps[:], Act.Copy)
                nc.sync.dma_start(
                    out_d[:].rearrange("(g p) d -> p g d", p=P),
                    stg[:].rearrange("p (g d) -> p g d", d=P))

    nc.compile()
    return nc


_cache2 = {}


def _bn_scale_shift(nc, mybir, smallp, red, gcol, bcol, idx):
    """Device-side BN coefficients from reduced stats: a = g*rstd, sh = b - mu*a."""
    f32 = mybir.dt.float32
    Alu = mybir.AluOpType
    Act = mybir.ActivationFunctionType
    mu = smallp.tile([P, 1], f32, tag=f"mu{idx}")
    nc.vector.tensor_scalar(mu[:], red[:, 0:1], 1.0 / N, None, Alu.mult)
    mu2 = smallp.tile([P, 1], f32, tag=f"mu2{idx}")
    nc.vector.tensor_tensor(mu2[:], mu[:], mu[:], Alu.mult)
    e2 = smallp.tile([P, 1], f32, tag=f"e2{idx}")
    nc.vector.tensor_scalar(e2[:], red[:, 1:2], 1.0 / N, None, Alu.mult)
    var = smallp.tile([P, 1], f32, tag=f"var{idx}")
    nc.vector.tensor_tensor(var[:], e2[:], mu2[:], Alu.subtract)
    vep = smallp.tile([P, 1], f32, tag=f"vep{idx}")
    nc.vector.tensor_scalar(vep[:], var[:], BN_EPS, None, Alu.add)
    std = smallp.tile([P, 1], f32, tag=f"std{idx}")
    nc.scalar.activation(std[:], vep[:], Act.Sqrt)
    rstd = smallp.tile([P, 1], f32, tag=f"rstd{idx}")
    nc.vector.reciprocal(rstd[:], std[:])
    a = smallp.tile([P, 1], f32, tag=f"a{idx}")
    nc.vector.tensor_tensor(a[:], gcol[:], rstd[:], Alu.mult)
    tmp = smallp.tile([P, 1], f32, tag=f"tmp{idx}")
    nc.vector.tensor_tensor(tmp[:], mu[:], a[:], Alu.mult)
    sh = smallp.tile([P, 1], f32, tag=f"sh{idx}")
    nc.vector.tensor_tensor(sh[:], bcol[:], tmp[:], Alu.subtract)
    return a, sh


def _build_phase2():
    """NEFF2: h1n = mask*relu(BN1(h1)); h2 = h1n @ w2.T; per-core stats of h2."""
    import concourse.bacc as bacc
    import concourse.mybir as mybir
    import concourse.tile as tile

    f32 = mybir.dt.float32
    Alu = mybir.AluOpType
    Act = mybir.ActivationFunctionType
    span = NTC * P

    nc = bacc.Bacc("TRN2", target_bir_lowering=False, debug=False,
                   num_devices=NCORES)
    nc.sbuf_top = min(nc.sbuf_top, 192 * 1024)

    def din(name, shape):
        return nc.dram_tensor(name, shape, f32, kind="ExternalInput")

    bf16 = mybir.dt.bfloat16
    h_d = nc.dram_tensor("h_in", [P, span], bf16, kind="ExternalInput")
    red_d = din("red", [P, 2])
    w2t_d = nc.dram_tensor("w2tb", [D, D], bf16, kind="ExternalInput")
    g1c_d = din("g1c", [P, 1]); b1c_d = din("b1c", [P, 1])
    maskb_d = nc.dram_tensor("maskb", [P, span], bf16, kind="ExternalInput")
    h_out_d = nc.dram_tensor("h_out", [P, span], bf16, kind="ExternalOutput")
    stat_out_d = nc.dram_tensor("stat_out", [P, 2], f32, kind="ExternalOutput")

    with tile.TileContext(nc) as tc:
        with (
            tc.tile_pool(name="consts", bufs=1) as cp,
            tc.tile_pool(name="psA", bufs=2, space="PSUM") as psA,
            tc.tile_pool(name="full", bufs=1) as fullp,
            tc.tile_pool(name="small", bufs=1) as smallp,
        ):
            def load_const(name, dram, shape, dt=f32):
                t = cp.tile(shape, dt, tag=f"c_{name}")
                nc.sync.dma_start(t[:], dram[:])
                return t

            h1 = load_const("h", h_d, [P, span], bf16)
            red = load_const("red", red_d, [P, 2])
            w2t = load_const("w2t", w2t_d, [D, D], bf16)
            g1c = load_const("g1c", g1c_d, [P, 1])
            b1c = load_const("b1c", b1c_d, [P, 1])
            maskb = load_const("maskb", maskb_d, [P, span], bf16)

            a, sh = _bn_scale_shift(nc, mybir, smallp, red, g1c, b1c, 0)
            hn = fullp.tile([P, span], bf16, tag="hn")
            nc.scalar.activation(hn[:], h1[:], Act.Relu, bias=sh[:], scale=a[:])
            nc.vector.tensor_tensor(hn[:], hn[:], maskb[:], Alu.mult)

            h2 = fullp.tile([P, span], bf16, tag="h2")
            for q0 in range(0, span, 512):
                qw = min(512, span - q0)
                ps = psA.tile([P, 512], f32, tag="psA")
                nc.tensor.matmul(ps[:, :qw], w2t[:], hn[:, q0:q0 + qw])
                nc.scalar.activation(h2[:, q0:q0 + qw], ps[:, :qw], Act.Copy)
            s_stat = smallp.tile([P, 2], f32, tag="stat2")
            nc.vector.reduce_sum(s_stat[:, 0:1], h2[:],
                                 axis=mybir.AxisListType.X)
            sq = fullp.tile([P, span], f32, tag="sq")
            nc.vector.tensor_tensor(sq[:], h2[:], h2[:], Alu.mult)
            nc.vector.reduce_sum(s_stat[:, 1:2], sq[:],
                                 axis=mybir.AxisListType.X)
            nc.sync.dma_start(stat_out_d[:], s_stat[:])
            nc.sync.dma_start(h_out_d[:], h2[:])

    nc.compile()
    return nc


def _build_phase3():
    """NEFF3: out = transpose(relu(BN2(h2)))."""
    import concourse.bacc as bacc
    import concourse.mybir as mybir
    import concourse.tile as tile

    f32 = mybir.dt.float32
    Act = mybir.ActivationFunctionType
    span = NTC * P

    nc = bacc.Bacc("TRN2", target_bir_lowering=False, debug=False,
                   num_devices=NCORES)
    nc.sbuf_top = min(nc.sbuf_top, 192 * 1024)

    def din(name, shape):
        return nc.dram_tensor(name, shape, f32, kind="ExternalInput")

    bf16 = mybir.dt.bfloat16
    h_d = nc.dram_tensor("h_in", [P, span], bf16, kind="ExternalInput")
    red_d = din("red", [P, 2])
    g2c_d = din("g2c", [P, 1]); b2c_d = din("b2c", [P, 1])
    ident_d = nc.dram_tensor("identb", [P, P], bf16, kind="ExternalInput")
    out_d = nc.dram_tensor("out", [span, D], f32, kind="ExternalOutput")

    with tile.TileContext(nc) as tc:
        with (
            tc.tile_pool(name="consts", bufs=1) as cp,
            tc.tile_pool(name="psA", bufs=2, space="PSUM") as psA,
            tc.tile_pool(name="full", bufs=1) as fullp,
            tc.tile_pool(name="small", bufs=1) as smallp,
        ):
            def load_const(name, dram, shape, dt=f32):
                t = cp.tile(shape, dt, tag=f"c_{name}")
                nc.sync.dma_start(t[:], dram[:])
                return t

            h2 = load_const("h", h_d, [P, span], bf16)
            red = load_const("red", red_d, [P, 2])
            g2c = load_const("g2c", g2c_d, [P, 1])
            b2c = load_const("b2c", b2c_d, [P, 1])
            ident = load_const("ident", ident_d, [P, P], bf16)

            a, sh = _bn_scale_shift(nc, mybir, smallp, red, g2c, b2c, 1)
            hn = fullp.tile([P, span], bf16, tag="hn")
            nc.scalar.activation(hn[:], h2[:], Act.Relu, bias=sh[:], scale=a[:])

            stg = fullp.tile([P, span], f32, tag="stg")
            for lt in range(NTC):
                sl = slice(lt * P, (lt + 1) * P)
                ps = psA.tile([P, P], bf16, tag="psA")
                nc.tensor.transpose(ps[:], hn[:, sl], ident[:])
                nc.scalar.activation(stg[:, sl], ps[:], Act.Copy)
            nc.sync.dma_start(
                out_d[:].rearrange("(g p) d -> p g d", p=P),
                stg[:].rearrange("p (g d) -> p g d", d=P))

    nc.compile()
    return nc


def _build_phase23():
    """Single second NEFF: every core gets the full (rotated) bf16 h1 with
    its own span at columns [0:span] and exact zeros appended for the 240
    padded nodes. BN1-apply + linear2 + global BN2 stats are computed
    replicated (mask-free, exact); BN2-apply + transpose + output only for
    the own span. Removes one NEFF launch and one host sync."""
    import concourse.bacc as bacc
    import concourse.mybir as mybir
    import concourse.tile as tile

    f32 = mybir.dt.float32
    bf16 = mybir.dt.bfloat16
    Alu = mybir.AluOpType
    Act = mybir.ActivationFunctionType
    span = NTC * P
    FULL = N2  # 10240 = 10000 real + 240 zero columns

    nc = bacc.Bacc("TRN2", target_bir_lowering=False, debug=False,
                   num_devices=NCORES)
    nc.sbuf_top = min(nc.sbuf_top, 192 * 1024)

    h_d = nc.dram_tensor("h_all", [P, FULL], bf16, kind="ExternalInput")
    red_d = nc.dram_tensor("red", [P, 2], f32, kind="ExternalInput")
    w2tb_d = nc.dram_tensor("w2tb", [D, D], bf16, kind="ExternalInput")
    g1c_d = nc.dram_tensor("g1c", [P, 1], f32, kind="ExternalInput")
    b1c_d = nc.dram_tensor("b1c", [P, 1], f32, kind="ExternalInput")
    g2c_d = nc.dram_tensor("g2c", [P, 1], f32, kind="ExternalInput")
    b2c_d = nc.dram_tensor("b2c", [P, 1], f32, kind="ExternalInput")
    ident_d = nc.dram_tensor("identb", [P, P], bf16, kind="ExternalInput")
    out_d = nc.dram_tensor("out", [span, D], f32, kind="ExternalOutput")

    with tile.TileContext(nc) as tc:
        with (
            tc.tile_pool(name="consts", bufs=1) as cp,
            tc.tile_pool(name="ps", bufs=2, space="PSUM") as psp,
            tc.tile_pool(name="full", bufs=1) as fullp,
            tc.tile_pool(name="small", bufs=1) as smallp,
        ):
            def load_const(name, dram, shape, dt=f32):
                t = cp.tile(shape, dt, tag=f"c_{name}")
                nc.sync.dma_start(t[:], dram[:])
                return t

            h1 = load_const("h", h_d, [P, FULL], bf16)
            red = load_const("red", red_d, [P, 2])
            w2tb = load_const("w2tb", w2tb_d, [D, D], bf16)
            g1c = load_const("g1c", g1c_d, [P, 1])
            b1c = load_const("b1c", b1c_d, [P, 1])
            g2c = load_const("g2c", g2c_d, [P, 1])
            b2c = load_const("b2c", b2c_d, [P, 1])
            ident = load_const("ident", ident_d, [P, P], bf16)

            a0, sh0 = _bn_scale_shift(nc, mybir, smallp, red, g1c, b1c, 0)
            # BN1-apply + relu in place (bf16); zero columns stay relu(sh0)
            # only in the 240 appended pad columns... they must be zeroed
            # for exact stats: recompute them as zeros via memset after.
            nc.scalar.activation(h1[:], h1[:], Act.Relu,
                                 bias=sh0[:], scale=a0[:])
            nc.vector.memset(h1[:, N:FULL], 0.0)

            # linear2 over the full width, bf16, w2 stationary
            h2 = fullp.tile([P, FULL], bf16, tag="h2")
            for q0 in range(0, FULL, 512):
                ps = psp.tile([P, 512], f32, tag="ps")
                for j in range(4):
                    nc.tensor.matmul(ps[:, j * P:(j + 1) * P], w2tb[:],
                                     h1[:, q0 + j * P:q0 + (j + 1) * P])
                nc.scalar.activation(h2[:, q0:q0 + 512], ps[:], Act.Copy)

            # global BN2 stats (exact: pad columns of h2 = W2 @ 0 = 0)
            s2 = smallp.tile([P, 2], f32, tag="s2")
            nc.vector.reduce_sum(s2[:, 0:1], h2[:], axis=mybir.AxisListType.X)
            sq = fullp.tile([P, FULL], f32, tag="sq")
            nc.vector.tensor_tensor(sq[:], h2[:], h2[:], Alu.mult)
            nc.vector.reduce_sum(s2[:, 1:2], sq[:], axis=mybir.AxisListType.X)
            a1, sh1 = _bn_scale_shift(nc, mybir, smallp, s2, g2c, b2c, 1)

            # BN2-apply + relu + transpose for the own span only
            hn2 = fullp.tile([P, span], bf16, tag="hn2")
            nc.scalar.activation(hn2[:], h2[:, :span], Act.Relu,
                                 bias=sh1[:], scale=a1[:])
            stg = fullp.tile([P, span], f32, tag="stg")
            for lt in range(NTC):
                sl = slice(lt * P, (lt + 1) * P)
                ps = psp.tile([P, P], bf16, tag="pst")
                nc.tensor.transpose(ps[:], hn2[:, sl], ident[:])
                nc.scalar.activation(stg[:, sl], ps[:], Act.Copy)
            nc.sync.dma_start(
                out_d[:].rearrange("(g p) d -> p g d", p=P),
                stg[:].rearrange("p (g d) -> p g d", d=P))

    nc.compile()
    return nc


def kernel(**inputs):
    global last_results
    from concourse.bass_utils import run_bass_kernel_spmd

    in_maps, meta = _host_prep(inputs)
    if meta not in _cache:
        _cache[meta] = _build(meta)
    if not KFUSE and KP23 and "p23" not in _cache2:
        _cache2["p23"] = _build_phase23()
    if not KFUSE and not KP23 and "p2" not in _cache2:
        _cache2["p2"] = _build_phase2()
        _cache2["p3"] = _build_phase3()
    nc1 = _cache[meta]
    nc2 = _cache2.get("p2"); nc3 = _cache2.get("p3")
    cores = list(range(NCORES))
    trace = bool(os.environ.get("KERNEL_TRACE"))

    n1_keys = ["xt", "xbt", "wxt", "wbt", "w1t", "linbb", "iotab",
               "xct", "srcw", "briw", "dstloc"]
    if KFUSE:
        n1_keys += ["w2t", "g1c", "b1c", "g2c", "b2c", "maskb", "ident"]
        in1 = [{k: in_maps[c][k] for k in n1_keys} for c in range(NCORES)]
        res1 = run_bass_kernel_spmd(nc1, in1, cores, trace=trace)
        last_results = (res1,)
        out = np.concatenate([res1.results[c]["out"] for c in range(NCORES)],
                             axis=0)
        return np.ascontiguousarray(out[:N])
    in1 = [{k: in_maps[c][k] for k in n1_keys} for c in range(NCORES)]
    res1 = run_bass_kernel_spmd(nc1, in1, cores, trace=trace)
    red1 = np.sum([res1.results[c]["stat_out"] for c in range(NCORES)], axis=0)
    if KP23:
        span = NTC * P
        hb = np.concatenate([res1.results[c]["h_out"] for c in range(NCORES)],
                            axis=1).astype(BF16)  # [P, 10240]
        zpad = np.zeros((P, N2 - N), BF16)
        in2 = []
        for c in range(NCORES):
            roll = np.concatenate(
                [hb[:, span * c:N], hb[:, :span * c], zpad], axis=1)
            in2.append({"h_all": np.ascontiguousarray(roll), "red": red1,
                        "w2tb": in_maps[c]["w2tb"],
                        "identb": in_maps[c]["identb"],
                        "g1c": in_maps[c]["g1c"], "b1c": in_maps[c]["b1c"],
                        "g2c": in_maps[c]["g2c"], "b2c": in_maps[c]["b2c"]})
        res2 = run_bass_kernel_spmd(_cache2["p23"], in2, cores, trace=trace)
        last_results = (res1, res2)
        out = np.concatenate([res2.results[c]["out"] for c in range(NCORES)],
                             axis=0)
        return np.ascontiguousarray(out[:N])
    in2 = [{"h_in": res1.results[c]["h_out"].astype(BF16), "red": red1,
            "w2tb": in_maps[c]["w2tb"], "g1c": in_maps[c]["g1c"],
            "b1c": in_maps[c]["b1c"], "maskb": in_maps[c]["maskb"]}
           for c in range(NCORES)]
    res2 = run_bass_kernel_spmd(nc2, in2, cores, trace=trace)
    red2 = np.sum([res2.results[c]["stat_out"] for c in range(NCORES)], axis=0)
    in3 = [{"h_in": res2.results[c]["h_out"], "red": red2,
            "g2c": in_maps[c]["g2c"], "b2c": in_maps[c]["b2c"],
            "identb": in_maps[c]["identb"]} for c in range(NCORES)]
    res3 = run_bass_kernel_spmd(nc3, in3, cores, trace=trace)

    last_results = (res1, res2, res3)
    out = np.concatenate([res3.results[c]["out"] for c in range(NCORES)], axis=0)
    return np.ascontiguousarray(out[:N])
